# revision 1
# baseline (speedup 1.0000x reference)
"""CLUB-NCE loss kernel for 8x Trainium2 NeuronCores (Bass/Tile).

Math (reference):
  hx = x @ W1x.T, hy = y @ W1y.T            [N, H]
  s[i,j]  = W2 . relu(hy[i] + hx[j] + b1) + b2
  T1[i,j] = softplus(s[i,j]); T0[i] = T1[i,i]
  lower = mean(T0) - (mean_i(logsumexp_j(T1[i,:])) - log N)
  upper = mean(T0) - mean(T1)

Sharding: y rows (i axis) split across 8 cores (64 rows each); x and MLP
params replicated. Each core computes its [64, 512] score block and emits
per-row partials (row sum of e^s, row sum of T1, diag e^s). Host combines.

Device design notes:
 - contraction dim k (=H=400) on partitions, tiled [128,128,128,16(+1)].
 - score row i is routed to PSUM partition i via a shifted one-hot
   stationary matrix: bsh[k] is [Pk, 128] with w2[k-chunk] at column 64,
   so lhsT = bsh[k][:, 64-ii : 96-ii] puts w2 in column ii.  All matmuls
   of a 32-row half accumulate into one [32, 512] PSUM bank; rows not
   owned by a matmul get exact +0.
 - the 16-deep k3 tail tiles of 8 consecutive rows are packed into one
   [128, 512] rhs (pairs of rows share a 32-partition block, scalars
   pre-paired by strided-rhs hy matmuls) and contracted by a single
   matmul with a banded stationary B3 (B3[p, 24 + 2*(p//32) +
   (p%32)//16] = w2[384 + p%16], window slid 8 columns per oct):
   25 matmuls per 8 rows — the exact (64*400)/128 = 200-matmul floor.
 - b1 is folded into the hy matmul as a virtual k=400 row.
 - PE warmup: dummy matmuls keep the tensor engine continuously busy
   from t~0.7us so it reaches full clock before real work, and filler
   dummies bridge the prologue->main handoff (any idle gap drops the
   PE clock for ~3us).
 - prologue is k-batch ordered (one batch per arriving input slab) so
   the PE never stalls mid-prologue.
 - two 32-row halves; half A's epilogue (Exp + softplus row sums on ACT,
   diag via identity-mask on DVE) hides under half B's matmuls.
 - per oct of rows: 24 row relu tiles on DVE (4x mode), 4 packed k3
   tiles split 1 DVE / 3 ACT.
 - x columns are rotated per core so the diag block sits at a fixed
   32-column window (row sums are permutation-invariant).
 - inputs: A slabs (x|w1x) on the SP queue pace the hx matmuls; B slabs
   (w1y|yt) and small tensors go through the Pool SWDGE queue to dodge
   the SP sequencer's ~650ns-per-DMA issue serialization.
 - host finishes: lse_i = log(N + rr_i), t0_i = log(1 + ed_i), means.
"""

import numpy as np

N = 512          # number of samples
D = 400          # feature dim
H = 400          # hidden dim
NCORES = 8
NL = N // NCORES  # 64 y-rows per core
NH = NL // 2      # 32 rows per half
KT = 4            # k tiles
KSZ = [128, 128, 128, 16]    # real k per tile (400 total)
KSZY = [128, 128, 128, 17]   # hy matmul k per tile (incl. bias row)
# consolidated input slabs, split in two pieces per k-tile so the hx
# matmuls can start before the hy-side data arrives:
#   A: x | w1x m0-2 | w1x m3-oct (8 copies of the 16 tail columns)
#   B: w1y m0-2 | w1y3 parity blocks | yt | bsh
CX, CW1X = 0, 512
ATOT = 1024
CW1Y, CYT = 0, 432
BTOT = 496
MSZ = [128, 128, 128, 128]   # H-tile partition sizes (m3 oct-duplicated)
NWARM = 1         # PE warmup dummy matmuls (anchors the p-state ramp)
NFILL = 0         # PE filler dummies between prologue and main loop
DROWS = 384       # dummy matmul free size


def _build_program(b2val: float, enable_asserts: bool = False):
    import concourse.bacc as bacc
    import concourse.mybir as mybir
    import concourse.tile as tile

    fp16 = mybir.dt.float16
    f32 = mybir.dt.float32
    AF = mybir.ActivationFunctionType
    ALU = mybir.AluOpType

    nc = bacc.Bacc(
        "TRN2",
        target_bir_lowering=False,
        debug=False,
        enable_asserts=enable_asserts,
    )

    slabA = nc.dram_tensor("slabA", [401, ATOT], fp16, kind="ExternalInput")
    slabB = nc.dram_tensor("slabB", [401, BTOT], fp16, kind="ExternalInput")
    # bsh one-hot tiles, k-tiles side by side in the free dim
    bshd = nc.dram_tensor("bshd", [128, 512], fp16, kind="ExternalInput")
    b3d = nc.dram_tensor("b3d", [128, 56], fp16, kind="ExternalInput")
    # x columns are rotated per core so the diag block sits at columns
    # [h*32, h*32+32) of half h; the mask is just a [32,32] identity
    maskd = nc.dram_tensor("maskd", [NH, NH], fp16, kind="ExternalInput")
    out_o = nc.dram_tensor("out_o", [NL, 4], f32, kind="ExternalOutput")

    with tile.TileContext(nc) as tc:
        with (
            tc.tile_pool(name="const", bufs=1) as cpool,
            tc.tile_pool(name="work", bufs=24) as wpool,
            tc.tile_pool(name="rq", bufs=4) as rqpool,
            tc.tile_pool(name="epi", bufs=2) as epool,
            tc.tile_pool(name="ppro", bufs=4, space="PSUM") as ppro,
            tc.tile_pool(name="phy", bufs=1, space="PSUM") as phy,
            tc.tile_pool(name="pmain", bufs=1, space="PSUM") as pmain,
            tc.tile_pool(name="pdum", bufs=1, space="PSUM") as pdum,
        ):
            # one table load (natural_log_exp_and_others: copy/relu/exp/ln)
            # hidden under the input DMAs instead of mid-epilogue
            nc.scalar.add_instruction(
                mybir.InstLoadActFuncSet(
                    name=nc.get_next_instruction_name(),
                    act_func_set_id=6,
                    engine=mybir.EngineType.Activation,
                    ins=[],
                    outs=[],
                )
            )

            # ---- input DMAs: A pieces first (feed hx), then B + B3 + mask ----
            sa_t, sb_t = [], []
            for k in range(KT):
                t = cpool.tile([KSZ[k], ATOT], fp16, name=f"slabA{k}")
                nc.sync.dma_start(
                    out=t, in_=slabA[k * 128 : k * 128 + KSZ[k], :]
                )
                sa_t.append(t)
            for k in range(KT):
                t = cpool.tile([KSZY[k], BTOT], fp16, name=f"slabB{k}")
                nc.gpsimd.dma_start(
                    out=t, in_=slabB[k * 128 : k * 128 + KSZY[k], :]
                )
                sb_t.append(t)
            bsh_t = cpool.tile([128, 512], fp16, name="bsh_t")
            nc.gpsimd.dma_start(out=bsh_t, in_=bshd[:, :])
            b3 = cpool.tile([128, 56], fp16, name="b3")
            nc.gpsimd.dma_start(out=b3, in_=b3d[:, :])
            mask = cpool.tile([NH, NH], fp16, name="mask")
            nc.gpsimd.dma_start(out=mask, in_=maskd[:, :])

            xt = [sa_t[k][:, CX : CX + N] for k in range(KT)]
            w1x = [sa_t[k][:, CW1X : CW1X + 512] for k in range(KT)]
            w1y = [sb_t[k][:, CW1Y : CW1Y + 384] for k in range(KT)]
            w1y3e = [sb_t[k][:, CW1Y + 384 : CW1Y + 416] for k in range(KT)]
            w1y3o = [sb_t[k][:, CW1Y + 400 : CW1Y + 432] for k in range(KT)]
            yt = [sb_t[k][:, CYT : CYT + NL] for k in range(KT)]
            bshl = [
                bsh_t[: KSZ[k], 128 * k : 128 * (k + 1)] for k in range(KT)
            ]

            # ---- PE warmup: keep the tensor engine busy from t~0 ----
            dumw = cpool.tile([128, 1], fp16, name="dumw")
            nc.vector.memset(dumw, 0.0)
            dumr = cpool.tile([128, DROWS], fp16, name="dumr")
            nc.vector.memset(dumr, 0.0)
            pd = pdum.tile([1, DROWS], f32, name="pd", tag="pd")

            b2t = cpool.tile([NH, 1], f32, name="b2t")
            nc.vector.memset(b2t, b2val)
            onet = cpool.tile([NH, 1], f32, name="onet")
            nc.vector.memset(onet, 1.0)
            out3 = cpool.tile([NL, 4], f32, name="out3")
            nc.vector.memset(out3, 0.0)

            def dummies(n):
                for _ in range(n):
                    nc.tensor.matmul(pd, lhsT=dumw, rhs=dumr,
                                     start=True, stop=True)

            dummies(NWARM)

            # ---- prologue, k-batch ordered: hy then hx per arriving slab ----
            pyall = phy.tile([128, 3 * NL], f32, name="pyall", tag="py")
            ph = [
                ppro.tile([MSZ[m], N], f32, name=f"ph{m}", tag="pp")
                for m in range(KT)
            ]
            # hx matmuls k-batched (one batch per arriving slab, no stalls;
            # the 4 ph banks are distinct so group interleaving is safe)
            for k in range(KT):
                for m in range(KT):
                    msl = (slice(m * 128, (m + 1) * 128) if m < 3
                           else slice(384, 512))
                    nc.tensor.matmul(
                        ph[m], lhsT=w1x[k][:, msl], rhs=xt[k],
                        start=(k == 0), stop=(k == KT - 1),
                    )
            # hy blocks share one PSUM bank: groups must be sequential per
            # block (same-bank interleaved start/stop corrupts accumulation)
            for m in range(3):
                msl = slice(m * 128, (m + 1) * 128)
                for k in range(KT):
                    nc.tensor.matmul(
                        pyall[:, m * NL : (m + 1) * NL],
                        lhsT=w1y[k][:, msl], rhs=yt[k],
                        start=(k == 0), stop=(k == KT - 1),
                    )
            # h3: per-partition-paired hy tail for the oct matmuls.
            # h3[32a+16p+m, t] = hy3[m, y-row 8t+2a+p] + b1[384+m]
            h3lo = ppro.tile([64, 8], f32, name="h3lo", tag="pp")
            h3hi = ppro.tile([64, 8], f32, name="h3hi", tag="pp")
            for a in range(4):
                dst = (h3lo if a < 2 else h3hi)[
                    32 * (a % 2) : 32 * (a % 2) + 32, :
                ]
                for par in range(2):
                    lh = w1y3e if par == 0 else w1y3o
                    for k in range(KT):
                        nc.tensor.matmul(
                            dst,
                            lhsT=lh[k],
                            rhs=yt[k][:, 2 * a + par : NL : 8],
                            start=(par == 0 and k == 0),
                            stop=(par == 1 and k == KT - 1),
                        )
            # psum -> sbuf staging, ordered for the fastest first-oct start:
            # DVE: hx0, hyb m-blocks, h3; ACT: hx1, hx2, hx3
            hx = [
                cpool.tile([MSZ[m], N], fp16, name=f"hx{m}")
                for m in range(KT)
            ]
            hyball = cpool.tile([128, 3 * NL], f32, name="hyball")
            h3s = cpool.tile([128, 8], f32, name="h3s")
            nc.vector.tensor_copy(out=hx[0], in_=ph[0])
            nc.vector.tensor_copy(out=hyball, in_=pyall)
            for m in (1, 2, 3):
                nc.scalar.activation(
                    out=hx[m], in_=ph[m], func=AF.Copy, bias=0.0, scale=1.0,
                )

            def hyb(m, i):  # per-partition scalar for H-tile m, row i
                return hyball[: MSZ[m], m * NL + i : m * NL + i + 1]

            dummies(NFILL)  # bridge prologue->main while copies drain

            # ---- main loop: two 32-row halves, octs of 8 rows ----
            def emit_oct(half, o):
                rq = rqpool.tile([128, N], fp16, name="rq", tag="rq")
                rks = []
                for rr in range(8):
                    i = half * NH + 8 * o + rr
                    for k in range(3):
                        r = wpool.tile([128, N], fp16, name=f"r{k}",
                                       tag=f"r{k}")
                        nc.vector.tensor_scalar(
                            out=r, in0=hx[k], scalar1=hyb(k, i), scalar2=0.0,
                            op0=ALU.add, op1=ALU.max,
                        )
                        rks.append(r)
                    if half == 0 and o == 0 and rr == 3:
                        nc.vector.tensor_copy(out=h3s[0:64, :], in_=h3lo)
                        nc.vector.tensor_copy(out=h3s[64:128, :], in_=h3hi)
                # k3 tails for rows 8o..8o+7 packed as 4 paired 32-blocks
                for a in range(4):
                    bsl = slice(32 * a, 32 * (a + 1))
                    if a == 0:
                        nc.vector.tensor_scalar(
                            out=rq[bsl, :], in0=hx[3][bsl, :],
                            scalar1=h3s[bsl, 4 * half + o : 4 * half + o + 1],
                            scalar2=0.0, op0=ALU.add, op1=ALU.max,
                        )
                    else:
                        nc.scalar.activation(
                            out=rq[bsl, :], in_=hx[3][bsl, :],
                            func=AF.Relu,
                            bias=h3s[bsl, 4 * half + o : 4 * half + o + 1],
                            scale=1.0,
                        )
                for rr in range(8):
                    ii = 8 * o + rr
                    for k in range(3):
                        nc.tensor.matmul(
                            ps_h[half], lhsT=bshl[k][:, 64 - ii : 96 - ii],
                            rhs=rks[3 * rr + k],
                            start=(o == 0 and rr == 0 and k == 0), stop=False,
                        )
                nc.tensor.matmul(
                    ps_h[half], lhsT=b3[:, 24 - 8 * o : 56 - 8 * o], rhs=rq,
                    start=False, stop=(o == NH // 8 - 1),
                )

            def emit_epilogue(half):
                osl = slice(half * NH, (half + 1) * NH)
                e2 = epool.tile([NH, N], fp16, name="e2", tag="e2")
                t1s = epool.tile([NH, N], fp16, name="t1s", tag="t1s")
                tmp = epool.tile([NH, NH], fp16, name="tmp", tag="tmp")
                # E = exp(s + b2); rr = row sums of E
                nc.scalar.activation(
                    out=e2, in_=ps_h[half], func=AF.Exp, bias=b2t, scale=1.0,
                    accum_out=out3[osl, 0:1],
                )
                # T1 = log(1 + E); rs = row sums of T1
                nc.scalar.activation(
                    out=t1s, in_=e2, func=AF.Ln, bias=onet, scale=1.0,
                    accum_out=out3[osl, 1:2],
                )
                # ed = diag(E): rotated x puts the diag block at a fixed
                # 32-column window
                nc.vector.tensor_tensor(
                    out=tmp,
                    in0=e2[:, half * NH : (half + 1) * NH], in1=mask,
                    op=ALU.mult,
                )
                nc.vector.reduce_sum(
                    out=out3[osl, 2:3], in_=tmp, axis=mybir.AxisListType.X
                )
                nc.sync.dma_start(out=out_o[osl, :], in_=out3[osl, :])

            ps_h = [
                pmain.tile([NH, N], f32, name=f"ps{h}", tag=f"ps{h}")
                for h in range(2)
            ]
            for o in range(NH // 8):
                emit_oct(0, o)
            for o in range(NH // 8):
                emit_oct(1, o)
                if o == 0:
                    emit_epilogue(0)
            emit_epilogue(1)

    nc.compile()
    return nc


def _make_in_maps(x, y, W1, b1, W2):
    f16 = np.float16
    slabA = np.zeros((401, ATOT), f16)
    slabB = np.zeros((401, BTOT), f16)
    w1xT = W1[:, :D].T.astype(f16)       # [D(k), H(m)]
    w1yT = W1[:, D:].T.astype(f16)
    slabA[:D, CW1X : CW1X + 384] = w1xT[:, :384]
    slabA[:D, CW1X + 384 : CW1X + 512] = np.tile(w1xT[:, 384:400], (1, 8))
    slabB[:D, CW1Y : CW1Y + 384] = w1yT[:, :384]
    slabB[400, CW1Y : CW1Y + 384] = b1[:384].astype(f16)
    # parity blocks: [384:416) = [w1y3 | 0], [400:432) = [0 | w1y3]
    slabB[:D, CW1Y + 384 : CW1Y + 400] = w1yT[:, 384:400]
    slabB[400, CW1Y + 384 : CW1Y + 400] = b1[384:400].astype(f16)
    slabB[:D, CW1Y + 416 : CW1Y + 432] = w1yT[:, 384:400]
    slabB[400, CW1Y + 416 : CW1Y + 432] = b1[384:400].astype(f16)
    bshp = np.zeros((128, 512), f16)
    for k in range(4):
        ksz = KSZ[k]
        bshp[:ksz, 128 * k + 64] = W2[0, 128 * k : 128 * k + ksz].astype(f16)
    b3p = np.zeros((128, 56), f16)
    p = np.arange(128)
    b3p[p, 24 + 2 * (p // 32) + (p % 32) // 16] = W2[0, 384 + (p % 16)].astype(f16)

    maskp = np.eye(NH, dtype=f16)
    xT = x.T.astype(f16)
    in_maps = []
    for c in range(NCORES):
        sa = slabA.copy()
        # rotate x columns so core c's diag block lands at columns [0, 64)
        sa[:D, CX : CX + N] = np.roll(xT, -c * NL, axis=1)
        sb = slabB.copy()
        sb[:D, CYT : CYT + NL] = y[c * NL : (c + 1) * NL, :].T.astype(f16)
        sb[400, CYT : CYT + NL] = 1.0
        in_maps.append({"slabA": sa, "slabB": sb, "bshd": bshp,
                        "b3d": b3p, "maskd": maskp})
    return in_maps


def _combine(results):
    rr = np.concatenate([r["out_o"][:, 0].astype(np.float64) for r in results])
    rs = np.concatenate([r["out_o"][:, 1].astype(np.float64) for r in results])
    ed = np.concatenate([r["out_o"][:, 2].astype(np.float64) for r in results])
    lse = np.log(np.float64(N) + rr)
    t0 = np.log1p(ed)
    t0_mean = t0.mean()
    lower = t0_mean - (lse.mean() - np.log(np.float64(N)))
    upper = t0_mean - rs.mean() / N
    return np.float32(lower), np.float32(upper)


def kernel(x_samples, y_samples, W1, b1, W2, b2, _trace=False):
    from concourse.bass_utils import run_bass_kernel_spmd

    nc = _build_program(float(np.float32(b2[0])))
    in_maps = _make_in_maps(
        np.asarray(x_samples, np.float32),
        np.asarray(y_samples, np.float32),
        np.asarray(W1, np.float32),
        np.asarray(b1, np.float32),
        np.asarray(W2, np.float32),
    )
    res = run_bass_kernel_spmd(
        nc, in_maps, core_ids=list(range(NCORES)), trace=_trace
    )
    out = _combine(res.results)
    if _trace:
        return out, res
    return out



# revision 12
# speedup vs baseline: 1.3625x; 1.3625x over previous
"""CLUB-NCE loss kernel for 8x Trainium2 NeuronCores (Bass/Tile).

Math (reference):
  hx = x @ W1x.T, hy = y @ W1y.T            [N, H]
  s[i,j]  = W2 . relu(hy[i] + hx[j] + b1) + b2
  T1[i,j] = softplus(s[i,j]); T0[i] = T1[i,i]
  lower = mean(T0) - (mean_i(logsumexp_j(T1[i,:])) - log N)
  upper = mean(T0) - mean(T1)

Sharding: y rows (i axis) split across 8 cores (64 rows each); x and MLP
params replicated. Each core computes its [64, 512] score block and emits
per-row partials (row sum of e^s, row sum of T1, diag e^s). Host combines.

Device design (v2 — transposed score matmuls):
 - relu tiles r[k](i) = relu(hx[k] + hy[i]) [128k, 512j] are produced on
   DVE/ACT/Pool (greedy load-balanced), then used as the matmul
   STATIONARY: matmul(out=[128j, 32i], lhsT=r[:, jb*128:...],
   rhs=bsh[k][:, 64-rr : 96-rr]) routes W2.r into output column rr.
   Output free size is 32, so each matmul is tiny; 4 j-blocks x 3 k-tiles
   x 32 rows + 4 packed-tail matmuls per half.
 - the 16-deep k3 tails of 8 rows are packed into one [128, 512] relu
   tile (hx tail oct-duplicated on partitions, h3 scalars packed
   16r+m -> hy3[m, row 8t+r]) and contracted by one matmul per j-block
   with a banded stationary b3t[p, 24 + p//16] = w2[384 + p%16].
 - scores live transposed [j, i]; per-half epilogue: ACT exp/ln into
   [128, 4*32] SBUF tiles, then ones-vector matmuls (output free size 1)
   produce row sums over j on PSUM partitions; diag via identity mask
   (x columns rotated per core so the diag block is at j in [0, 64)).
 - b1 folded into the hy matmul as a virtual k=400 row.
 - prologue is k-batch ordered (one batch per arriving input slab);
   warmup dummies anchor the PE p-state ramp.
 - A slabs (x|w1x) on the SP HWDGE queue; B slabs (w1y|w1y3|yt) and the
   merged consts tile (bsh|b3t|mask) on the Pool SWDGE queue.
 - host finishes: lse_i = log(N + rr_i), t0_i = log(1 + ed_i), means.
"""

import numpy as np

N = 512          # number of samples
D = 400          # feature dim
H = 400          # hidden dim
NCORES = 8
NL = N // NCORES  # 64 y-rows per core
NH = NL // 2      # 32 rows per half
KT = 4            # k tiles
KSZ = [128, 128, 128, 16]    # real k per tile (400 total)
KSZY = [128, 128, 128, 17]   # hy matmul k per tile (incl. bias row)
# consolidated input slabs:
#   A: x | w1x m0-2 | w1x m3-oct (8 copies of the 16 tail columns)
#   B: w1y m0-2 | w1y3 parity blocks | yt
CX, CW1X = 0, 512
ATOT = 1024
CW1Y, CYT = 0, 432
BTOT = 496
# consts tile: bsh (3x128) | b3t (56) | mask (32, at partitions 0:32)
CBSH, CB3, CMSK = 0, 384, 440
CTOT = 472
MSZ = [128, 128, 128, 128]   # H-tile partition sizes (m3 oct-duplicated)
NWARM = 1         # PE warmup dummy matmuls (anchors the p-state ramp)
DROWS = 384       # dummy matmul free size

# relu-tile engine split: greedy balance by per-tile engine cost (ns)
ENG_COST = {"D": 163.0, "A": 519.0, "P": 806.0}
# initial load offsets (ns): ACT pays epilogue+copies, Pool late start
ENG_SEED = {"D": 0.0, "A": 2500.0, "P": 1500.0}


def _relu_schedule():
    """Greedy assignment of the 200 relu tiles to engines."""
    load = dict(ENG_SEED)
    out = []
    for _ in range(2 * 4 * 25):  # halves x octs x (24 row tiles + rq)
        e = min(load, key=lambda x: load[x] + ENG_COST[x])
        load[e] += ENG_COST[e]
        out.append(e)
    return out


def _build_program(b2val: float, enable_asserts: bool = False):
    import concourse.bacc as bacc
    import concourse.mybir as mybir
    import concourse.tile as tile

    fp16 = mybir.dt.float16
    f32 = mybir.dt.float32
    AF = mybir.ActivationFunctionType
    ALU = mybir.AluOpType

    nc = bacc.Bacc(
        "TRN2",
        target_bir_lowering=False,
        debug=False,
        enable_asserts=enable_asserts,
    )

    slabA = nc.dram_tensor("slabA", [401, ATOT], fp16, kind="ExternalInput")
    slabB = nc.dram_tensor("slabB", [401, BTOT], fp16, kind="ExternalInput")
    constd = nc.dram_tensor("constd", [128, CTOT], fp16, kind="ExternalInput")
    out_o = nc.dram_tensor("out_o", [NL, 4], f32, kind="ExternalOutput")

    sched = _relu_schedule()

    with tile.TileContext(nc) as tc:
        with (
            tc.tile_pool(name="const", bufs=1) as cpool,
            tc.tile_pool(name="work", bufs=24) as wpool,
            tc.tile_pool(name="rq", bufs=4) as rqpool,
            tc.tile_pool(name="epi", bufs=2) as epool,
            tc.tile_pool(name="psum", bufs=8, space="PSUM") as pp,
        ):
            # one table load (natural_log_exp_and_others: copy/relu/exp/ln)
            nc.scalar.add_instruction(
                mybir.InstLoadActFuncSet(
                    name=nc.get_next_instruction_name(),
                    act_func_set_id=6,
                    engine=mybir.EngineType.Activation,
                    ins=[],
                    outs=[],
                )
            )

            # ---- input DMAs: A pieces on SP (feed hx), B + consts on Pool
            sa_t, sb_t = [], []
            for k in range(KT):
                t = cpool.tile([KSZ[k], ATOT], fp16, name=f"slabA{k}")
                nc.sync.dma_start(
                    out=t, in_=slabA[k * 128 : k * 128 + KSZ[k], :]
                )
                sa_t.append(t)
            for k in range(KT):
                t = cpool.tile([KSZY[k], BTOT], fp16, name=f"slabB{k}")
                nc.gpsimd.dma_start(
                    out=t, in_=slabB[k * 128 : k * 128 + KSZY[k], :]
                )
                sb_t.append(t)
            cons = cpool.tile([128, CTOT], fp16, name="cons")
            nc.gpsimd.dma_start(out=cons, in_=constd[:, :])

            xt = [sa_t[k][:, CX : CX + N] for k in range(KT)]
            w1x = [sa_t[k][:, CW1X : CW1X + 512] for k in range(KT)]
            w1y = [sb_t[k][:, CW1Y : CW1Y + 384] for k in range(KT)]
            w1y3e = [sb_t[k][:, CW1Y + 384 : CW1Y + 416] for k in range(KT)]
            w1y3o = [sb_t[k][:, CW1Y + 400 : CW1Y + 432] for k in range(KT)]
            yt = [sb_t[k][:, CYT : CYT + NL] for k in range(KT)]
            bshl = [
                cons[: KSZ[k], CBSH + 128 * k : CBSH + 128 * (k + 1)]
                for k in range(3)
            ]
            b3t = cons[:, CB3 : CB3 + 56]
            # two stacked identity copies so each half's diag extraction
            # reads a mask at its own base partition (32h)
            maskh = [cons[32 * h : 32 * h + NH, CMSK : CMSK + NH]
                     for h in range(2)]

            # ---- small consts ----
            dumw = cpool.tile([128, 1], fp16, name="dumw")
            nc.vector.memset(dumw, 0.0)
            dumr = cpool.tile([128, DROWS], fp16, name="dumr")
            nc.vector.memset(dumr, 0.0)
            b2t = cpool.tile([128, 1], f32, name="b2t")
            nc.vector.memset(b2t, b2val)
            onef = cpool.tile([128, 1], f32, name="onef")
            nc.vector.memset(onef, 1.0)
            one16 = cpool.tile([128, 1], fp16, name="one16")
            nc.vector.memset(one16, 1.0)
            out3 = cpool.tile([NL, 4], f32, name="out3")
            nc.vector.memset(out3, 0.0)

            # ---- PSUM ring: 1 dummy + 5 prologue + 8 pso + 2 pout, bufs=8
            # per tag; tags share one ring via tag="pp" (full-bank tiles)
            def pbank(name):
                return pp.tile([128, 512], f32, name=name, tag="pp")

            pd = pbank("pd")

            def dummies(n, free=DROWS):
                for _ in range(n):
                    nc.tensor.matmul(pd[:1, :free], lhsT=dumw, rhs=dumr[:, :free],
                                     start=True, stop=True)

            dummies(NWARM)

            # ---- prologue, k-batch ordered: hx then hy per arriving slab
            ph = [pbank(f"ph{m}") for m in range(KT)]
            pyh = pbank("pyh")  # cols 0:192 hy m-blocks, 192:200 h3
            for k in range(KT):
                for m in range(KT):
                    msl = (slice(m * 128, (m + 1) * 128) if m < 3
                           else slice(384, 512))
                    nc.tensor.matmul(
                        ph[m][:, :N], lhsT=w1x[k][:, msl], rhs=xt[k],
                        start=(k == 0), stop=(k == KT - 1),
                    )
            # hy blocks + h3 share one PSUM bank: groups sequential
            for m in range(3):
                msl = slice(m * 128, (m + 1) * 128)
                for k in range(KT):
                    nc.tensor.matmul(
                        pyh[:, m * NL : (m + 1) * NL],
                        lhsT=w1y[k][:, msl], rhs=yt[k],
                        start=(k == 0), stop=(k == KT - 1),
                    )
            # h3: per-partition-paired hy tail, packed like the baseline:
            # h3[32a+16p+m, t] = hy3[m, y-row 8t+2a+p] + b1[384+m].
            # h3lo (a=0,1) at pyh cols 192:200, h3hi (a=2,3) at 200:208.
            for a in range(4):
                csl = slice(192, 200) if a < 2 else slice(200, 208)
                psl = slice(32 * (a % 2), 32 * (a % 2) + 32)
                for par in range(2):
                    lh = w1y3e if par == 0 else w1y3o
                    for k in range(KT):
                        nc.tensor.matmul(
                            pyh[psl, csl],
                            lhsT=lh[k],
                            rhs=yt[k][:, 2 * a + par : NL : 8],
                            start=(par == 0 and k == 0),
                            stop=(par == 1 and k == KT - 1),
                        )
            # psum -> sbuf staging: DVE hx0 + hy/h3; ACT hx1..3
            hx = [
                cpool.tile([MSZ[m], N], fp16, name=f"hx{m}")
                for m in range(KT)
            ]
            hyball = cpool.tile([128, 192], f32, name="hyball")
            h3s = cpool.tile([128, 8], f32, name="h3s")
            nc.vector.tensor_copy(out=hx[0], in_=ph[0][:, :N])
            nc.vector.tensor_copy(out=hyball, in_=pyh[:, :192])
            nc.vector.tensor_copy(out=h3s[0:64, :], in_=pyh[0:64, 192:200])
            nc.vector.tensor_copy(out=h3s[64:128, :], in_=pyh[0:64, 200:208])
            for m in (1, 2, 3):
                nc.scalar.activation(
                    out=hx[m], in_=ph[m][:, :N], func=AF.Copy, bias=0.0,
                    scale=1.0,
                )

            def hyb(m, i):  # per-partition scalar for H-tile m, row i
                return hyball[: MSZ[m], m * NL + i : m * NL + i + 1]

            def h3col(t):
                return h3s[:, t : t + 1]

            # ---- main loop: two 32-row halves, octs of 8 rows ----
            pso = [[pbank(f"ps{h}{jb}") for jb in range(4)] for h in range(2)]

            def relu_tile(eng, out, in_, scalar):
                if eng == "D":
                    nc.vector.tensor_scalar(
                        out=out, in0=in_, scalar1=scalar, scalar2=0.0,
                        op0=ALU.add, op1=ALU.max,
                    )
                elif eng == "P":
                    nc.gpsimd.tensor_scalar(
                        out=out, in0=in_, scalar1=scalar, scalar2=0.0,
                        op0=ALU.add, op1=ALU.max,
                    )
                else:
                    nc.scalar.activation(
                        out=out, in_=in_, func=AF.Relu, bias=scalar,
                        scale=1.0,
                    )

            si = [0]  # schedule cursor

            def next_eng():
                e = sched[si[0] % len(sched)]
                si[0] += 1
                return e

            def emit_oct(half, o):
                for k in range(3):
                    for r in range(8):
                        rr = 8 * o + r
                        i = half * NH + rr
                        rt = wpool.tile([128, N], fp16, name="rt", tag="rt")
                        relu_tile(next_eng(), rt, hx[k], hyb(k, i))
                        for jb in range(4):
                            nc.tensor.matmul(
                                pso[half][jb][:, :NH],
                                lhsT=rt[:, 128 * jb : 128 * (jb + 1)],
                                rhs=bshl[k][:, 64 - rr : 96 - rr],
                                start=(o == 0 and k == 0 and r == 0),
                                stop=False,
                            )
                # packed k3 tail for the oct's 8 rows
                rq = rqpool.tile([128, N], fp16, name="rq", tag="rq")
                relu_tile(next_eng(), rq, hx[3], h3col(4 * half + o))
                for jb in range(4):
                    nc.tensor.matmul(
                        pso[half][jb][:, :NH],
                        lhsT=rq[:, 128 * jb : 128 * (jb + 1)],
                        rhs=b3t[:, 24 - 8 * o : 56 - 8 * o],
                        start=False, stop=(o == NH // 8 - 1),
                    )

            epi = {}

            def emit_epilogue_a(half):
                # ACT + PE part: exp/ln tiles + row-sum matmuls
                e2 = epool.tile([128, 128], fp16, name="e2", tag="e2")
                t1 = epool.tile([128, 128], fp16, name="t1", tag="t1")
                # E^T blocks = exp(s + b2), [128j, 32i] per j-block
                for jb in range(4):
                    nc.scalar.activation(
                        out=e2[:, 32 * jb : 32 * (jb + 1)],
                        in_=pso[half][jb][:, :NH],
                        func=AF.Exp, bias=b2t, scale=1.0,
                    )
                # T1^T = log(1 + E^T)
                nc.scalar.activation(
                    out=t1, in_=e2, func=AF.Ln, bias=onef, scale=1.0,
                )
                # row sums over j via ones-vector matmuls (free size 1)
                po = pbank(f"po{half}")
                for jb in range(4):
                    nc.tensor.matmul(
                        po[:NH, 0:1], lhsT=e2[:, 32 * jb : 32 * (jb + 1)],
                        rhs=one16, start=(jb == 0), stop=(jb == 3),
                    )
                for jb in range(4):
                    nc.tensor.matmul(
                        po[:NH, 1:2], lhsT=t1[:, 32 * jb : 32 * (jb + 1)],
                        rhs=one16, start=(jb == 0), stop=(jb == 3),
                    )
                epi[half] = (e2, po)

            def emit_epilogue_b(half):
                # DVE + DMA part: diag, out3 assembly, output DMA
                e2, po = epi[half]
                osl = slice(half * NH, (half + 1) * NH)
                tmp = epool.tile([NL, NH], fp16, name="tmp", tag="tmp")
                # ed = diag(E): rotated x puts the diag block at
                # j in [32h, 32h+32), i.e. partitions 32h.. of j-block 0
                nc.vector.tensor_tensor(
                    out=tmp[osl, :],
                    in0=e2[half * NH : (half + 1) * NH, 0:NH],
                    in1=maskh[half],
                    op=ALU.mult,
                )
                nc.vector.reduce_sum(
                    out=out3[osl, 2:3], in_=tmp[osl, :],
                    axis=mybir.AxisListType.X,
                )
                nc.vector.tensor_copy(out=out3[osl, 0:2], in_=po[:NH, 0:2])
                nc.sync.dma_start(out=out_o[osl, :], in_=out3[osl, :])

            for o in range(NH // 8):
                emit_oct(0, o)
            for o in range(NH // 8):
                emit_oct(1, o)
                if o == 0:
                    emit_epilogue_a(0)
                if o == 2:
                    emit_epilogue_b(0)
            emit_epilogue_a(1)
            emit_epilogue_b(1)

    nc.compile()
    return nc


def _make_in_maps(x, y, W1, b1, W2):
    f16 = np.float16
    slabA = np.zeros((401, ATOT), f16)
    slabB = np.zeros((401, BTOT), f16)
    w1xT = W1[:, :D].T.astype(f16)       # [D(k), H(m)]
    w1yT = W1[:, D:].T.astype(f16)
    slabA[:D, CW1X : CW1X + 384] = w1xT[:, :384]
    slabA[:D, CW1X + 384 : CW1X + 512] = np.tile(w1xT[:, 384:400], (1, 8))
    slabB[:D, CW1Y : CW1Y + 384] = w1yT[:, :384]
    slabB[400, CW1Y : CW1Y + 384] = b1[:384].astype(f16)
    # parity blocks: [384:416) = [w1y3 | 0], [400:432) = [0 | w1y3]
    slabB[:D, CW1Y + 384 : CW1Y + 400] = w1yT[:, 384:400]
    slabB[400, CW1Y + 384 : CW1Y + 400] = b1[384:400].astype(f16)
    slabB[:D, CW1Y + 416 : CW1Y + 432] = w1yT[:, 384:400]
    slabB[400, CW1Y + 416 : CW1Y + 432] = b1[384:400].astype(f16)

    consts = np.zeros((128, CTOT), f16)
    for k in range(3):
        consts[:128, CBSH + 128 * k + 64] = W2[0, 128 * k : 128 * (k + 1)].astype(f16)
    p = np.arange(128)
    consts[p, CB3 + 24 + 2 * (p // 32) + (p % 32) // 16] = W2[0, 384 + (p % 16)].astype(f16)
    consts[:NH, CMSK : CMSK + NH] = np.eye(NH, dtype=f16)
    consts[NH : 2 * NH, CMSK : CMSK + NH] = np.eye(NH, dtype=f16)

    xT = x.T.astype(f16)
    in_maps = []
    for c in range(NCORES):
        sa = slabA.copy()
        # rotate x columns so core c's diag block lands at columns [0, 64)
        sa[:D, CX : CX + N] = np.roll(xT, -c * NL, axis=1)
        sb = slabB.copy()
        sb[:D, CYT : CYT + NL] = y[c * NL : (c + 1) * NL, :].T.astype(f16)
        sb[400, CYT : CYT + NL] = 1.0
        in_maps.append({"slabA": sa, "slabB": sb, "constd": consts})
    return in_maps


def _combine(results):
    rr = np.concatenate([r["out_o"][:, 0].astype(np.float64) for r in results])
    rs = np.concatenate([r["out_o"][:, 1].astype(np.float64) for r in results])
    ed = np.concatenate([r["out_o"][:, 2].astype(np.float64) for r in results])
    lse = np.log(np.float64(N) + rr)
    t0 = np.log1p(ed)
    t0_mean = t0.mean()
    lower = t0_mean - (lse.mean() - np.log(np.float64(N)))
    upper = t0_mean - rs.mean() / N
    return np.float32(lower), np.float32(upper)


def kernel(x_samples, y_samples, W1, b1, W2, b2, _trace=False):
    from concourse.bass_utils import run_bass_kernel_spmd

    nc = _build_program(float(np.float32(b2[0])))
    in_maps = _make_in_maps(
        np.asarray(x_samples, np.float32),
        np.asarray(y_samples, np.float32),
        np.asarray(W1, np.float32),
        np.asarray(b1, np.float32),
        np.asarray(W2, np.float32),
    )
    res = run_bass_kernel_spmd(
        nc, in_maps, core_ids=list(range(NCORES)), trace=_trace
    )
    out = _combine(res.results)
    if _trace:
        return out, res
    return out


# revision 47
# speedup vs baseline: 1.4140x; 1.0378x over previous
"""CLUB-NCE loss kernel for 8x Trainium2 NeuronCores (Bass/Tile).

Math (reference):
  hx = x @ W1x.T, hy = y @ W1y.T            [N, H]
  s[i,j]  = W2 . relu(hy[i] + hx[j] + b1) + b2
  T1[i,j] = softplus(s[i,j]); T0[i] = T1[i,i]
  lower = mean(T0) - (mean_i(logsumexp_j(T1[i,:])) - log N)
  upper = mean(T0) - mean(T1)

Sharding: y rows (i axis) split across 8 cores (64 rows each); x and MLP
params replicated. Each core computes its [64, 512] score block and emits
per-row partials (row sum of e^s, row sum of T1, diag e^s). Host combines.

Device design (v2 — transposed score matmuls):
 - relu tiles r[k](i) = relu(hx[k] + hy[i]) [128k, 512j] are produced on
   DVE/ACT/Pool (greedy load-balanced), then used as the matmul
   STATIONARY: matmul(out=[128j, 32i], lhsT=r[:, jb*128:...],
   rhs=bsh[k][:, 64-rr : 96-rr]) routes W2.r into output column rr.
   Output free size is 32, so each matmul is tiny; 4 j-blocks x 3 k-tiles
   x 32 rows + 4 packed-tail matmuls per half.
 - the 16-deep k3 tails of 8 rows are packed into one [128, 512] relu
   tile (hx tail oct-duplicated on partitions, h3 scalars packed
   16r+m -> hy3[m, row 8t+r]) and contracted by one matmul per j-block
   with a banded stationary b3t[p, 24 + p//16] = w2[384 + p%16].
 - scores live transposed [j, i]; per-half epilogue: ACT exp/ln into
   [128, 4*32] SBUF tiles, then ones-vector matmuls (output free size 1)
   produce row sums over j on PSUM partitions; diag via identity mask
   (x columns rotated per core so the diag block is at j in [0, 64)).
 - b1 folded into the hy matmul as a virtual k=400 row.
 - prologue is k-batch ordered (one batch per arriving input slab);
   warmup dummies anchor the PE p-state ramp.
 - A slabs (x|w1x) on the SP HWDGE queue; B slabs (w1y|w1y3|yt) and the
   merged consts tile (bsh|b3t|mask) on the Pool SWDGE queue.
 - host finishes: lse_i = log(N + rr_i), t0_i = log(1 + ed_i), means.
"""

import numpy as np

N = 512          # number of samples
D = 400          # feature dim
H = 400          # hidden dim
NCORES = 8
NL = N // NCORES  # 64 y-rows per core
NH = NL // 2      # 32 rows per half
KT = 4            # k tiles
KSZ = [128, 128, 128, 16]    # real k per tile (400 total)
KSZY = [128, 128, 128, 17]   # hy matmul k per tile (incl. bias row)
# consolidated input slabs:
#   A: x | w1x m0-2 | w1x m3-oct (8 copies of the 16 tail columns)
#   B: w1y m0-2 | w1y3 parity blocks | yt
CX, CW1X = 0, 512
ATOT = 1024
CW1Y, CYT = 0, 432
BTOT = 496
# consts tile: bsh (3x128) | b3t (56) | mask (32, at partitions 0:32)
CBSH, CB3, CMSK = 0, 384, 440
CTOT = 472
MSZ = [128, 128, 128, 128]   # H-tile partition sizes (m3 oct-duplicated)
NWARM = 1         # PE warmup dummy matmuls (anchors the p-state ramp)
DROWS = 384       # dummy matmul free size

import os as _os

# relu-tile engine split: greedy balance by per-tile engine cost (ns,
# measured in TimelineSim: free-size*cycle + init/2 [+ Q7 launch])
ENG_COST = {"D": 194.0, "A": 630.0, "P": 806.0}
# initial load offsets (ns): ACT pays the mid-window epilogue-A AND the
# terminal exp/ln chain (its stream must end first); Pool starts late
# (SWDGE prologue + hx copies)
ENG_SEED = {
    "D": float(_os.environ.get("K_SEED_D", 0.0)),
    "A": float(_os.environ.get("K_SEED_A", 1000.0)),
    "P": float(_os.environ.get("K_SEED_P", 200.0)),
}
K_RQ_EARLY = int(_os.environ.get("K_RQ_EARLY", 1))
K_DIRECT = int(_os.environ.get("K_DIRECT", 0))
K_HXSPLIT = _os.environ.get("K_HXSPLIT", "a")  # a: hx0 DVE + hx123 ACT
                                               # b: hx01 ACT + hx23 DVE-woven


def _relu_schedule():
    """Greedy assignment of the 200 relu tiles to engines."""
    load = dict(ENG_SEED)
    out = []
    for _ in range(2 * 4 * 25):  # halves x octs x (24 row tiles + rq)
        e = min(load, key=lambda x: load[x] + ENG_COST[x])
        load[e] += ENG_COST[e]
        out.append(e)
    return out


def _build_program(b2val: float, enable_asserts: bool = False):
    import concourse.bacc as bacc
    import concourse.mybir as mybir
    import concourse.tile as tile

    fp16 = mybir.dt.float16
    f32 = mybir.dt.float32
    AF = mybir.ActivationFunctionType
    ALU = mybir.AluOpType

    nc = bacc.Bacc(
        "TRN2",
        target_bir_lowering=False,
        debug=False,
        enable_asserts=enable_asserts,
    )

    slabA = nc.dram_tensor("slabA", [401, ATOT], fp16, kind="ExternalInput")
    slabB = nc.dram_tensor("slabB", [401, BTOT], fp16, kind="ExternalInput")
    constd = nc.dram_tensor("constd", [128, CTOT], fp16, kind="ExternalInput")
    out_o = nc.dram_tensor("out_o", [NL, 4], f32, kind="ExternalOutput")

    sched = _relu_schedule()

    with tile.TileContext(nc) as tc:
        with (
            tc.tile_pool(name="const", bufs=1) as cpool,
            tc.tile_pool(name="work", bufs=48) as wpool,
            tc.tile_pool(name="rq", bufs=4) as rqpool,
            tc.tile_pool(name="epi", bufs=2) as epool,
            tc.tile_pool(name="psum", bufs=8, space="PSUM") as pp,
        ):
            # one table load (natural_log_exp_and_others: copy/relu/exp/ln)
            nc.scalar.add_instruction(
                mybir.InstLoadActFuncSet(
                    name=nc.get_next_instruction_name(),
                    act_func_set_id=6,
                    engine=mybir.EngineType.Activation,
                    ins=[],
                    outs=[],
                )
            )

            # ---- input DMAs: A pieces on SP (feed hx), B + consts on Pool
            sa_t, sb_t = [], []
            for k in range(KT):
                t = cpool.tile([KSZ[k], ATOT], fp16, name=f"slabA{k}")
                nc.sync.dma_start(
                    out=t, in_=slabA[k * 128 : k * 128 + KSZ[k], :]
                )
                sa_t.append(t)
            for k in range(KT):
                t = cpool.tile([KSZY[k], BTOT], fp16, name=f"slabB{k}")
                # B0/B1 on Pool SWDGE (parallel issue channel), B2/B3 on
                # the SP queue behind the A slabs: Pool's engine frees by
                # ~3us and the last B piece still lands earlier
                q = nc.gpsimd if k < 3 else nc.sync
                q.dma_start(
                    out=t, in_=slabB[k * 128 : k * 128 + KSZY[k], :]
                )
                sb_t.append(t)
            # consts ride the SP queue after the A slabs (keeps Pool's
            # SWDGE queue short so the B slabs land earlier)
            cons = cpool.tile([128, CTOT], fp16, name="cons")
            nc.sync.dma_start(out=cons, in_=constd[:, :])

            xt = [sa_t[k][:, CX : CX + N] for k in range(KT)]
            w1x = [sa_t[k][:, CW1X : CW1X + 512] for k in range(KT)]
            w1y = [sb_t[k][:, CW1Y : CW1Y + 384] for k in range(KT)]
            w1y3e = [sb_t[k][:, CW1Y + 384 : CW1Y + 416] for k in range(KT)]
            w1y3o = [sb_t[k][:, CW1Y + 400 : CW1Y + 432] for k in range(KT)]
            yt = [sb_t[k][:, CYT : CYT + NL] for k in range(KT)]
            bshl = [
                cons[: KSZ[k], CBSH + 128 * k : CBSH + 128 * (k + 1)]
                for k in range(3)
            ]
            b3t = cons[:, CB3 : CB3 + 56]
            # two stacked identity copies so each half's diag extraction
            # reads a mask at its own base partition (32h)
            maskh = [cons[32 * h : 32 * h + NH, CMSK : CMSK + NH]
                     for h in range(2)]

            # ---- small consts ----
            dumw = cpool.tile([128, 1], fp16, name="dumw")
            nc.vector.memset(dumw, 0.0)
            dumr = cpool.tile([128, DROWS], fp16, name="dumr")
            nc.vector.memset(dumr, 0.0)
            b2t = cpool.tile([128, 1], f32, name="b2t")
            nc.vector.memset(b2t, b2val)
            onef = cpool.tile([128, 1], f32, name="onef")
            nc.vector.memset(onef, 1.0)
            one16 = cpool.tile([128, 1], fp16, name="one16")
            nc.vector.memset(one16, 1.0)
            out3 = cpool.tile([NL, 4], f32, name="out3")
            nc.vector.memset(out3, 0.0)

            # ---- PSUM ring: 1 dummy + 5 prologue + 8 pso + 2 pout, bufs=8
            # per tag; tags share one ring via tag="pp" (full-bank tiles)
            def pbank(name):
                return pp.tile([128, 512], f32, name=name, tag="pp")

            pd = pbank("pd")

            def dummies(n, free=DROWS):
                for _ in range(n):
                    nc.tensor.matmul(pd[:1, :free], lhsT=dumw, rhs=dumr[:, :free],
                                     start=True, stop=True)

            dummies(NWARM)

            # ---- prologue, k-batch ordered: hx then hy per arriving slab
            ph = [pbank(f"ph{m}") for m in range(KT)]
            pyh = pbank("pyh")  # cols 0:192 hy m-blocks, 192:208 h3

            def hx_batch(k, ms):
                for m in ms:
                    msl = (slice(m * 128, (m + 1) * 128) if m < 3
                           else slice(384, 512))
                    nc.tensor.matmul(
                        ph[m][:, :N], lhsT=w1x[k][:, msl], rhs=xt[k],
                        start=(k == 0), stop=(k == KT - 1),
                    )

            # k0..k2 batches in full; then ONLY m0 of the k3 batch so the
            # hx0 copy (and with it the first relu tiles) unblocks before
            # the rest of the prologue drains
            for k in range(3):
                hx_batch(k, range(KT))
            hx_batch(3, [0])
            # hy blocks + h3 share one PSUM bank: groups sequential
            for m in range(3):
                msl = slice(m * 128, (m + 1) * 128)
                for k in range(KT):
                    nc.tensor.matmul(
                        pyh[:, m * NL : (m + 1) * NL],
                        lhsT=w1y[k][:, msl], rhs=yt[k],
                        start=(k == 0), stop=(k == KT - 1),
                    )
            # h3: per-partition-paired hy tail, packed like the baseline:
            # h3[32a+16p+m, t] = hy3[m, y-row 8t+2a+p] + b1[384+m].
            # h3lo (a=0,1) at pyh cols 192:200, h3hi (a=2,3) at 200:208.
            for a in range(4):
                csl = slice(192, 200) if a < 2 else slice(200, 208)
                psl = slice(32 * (a % 2), 32 * (a % 2) + 32)
                for par in range(2):
                    lh = w1y3e if par == 0 else w1y3o
                    for k in range(KT):
                        nc.tensor.matmul(
                            pyh[psl, csl],
                            lhsT=lh[k],
                            rhs=yt[k][:, 2 * a + par : NL : 8],
                            start=(par == 0 and k == 0),
                            stop=(par == 1 and k == KT - 1),
                        )
            # finish the hx k3 batch after hy/h3 so hx0 unblocked early
            hx_batch(3, [1, 2, 3])

            # psum -> sbuf staging: hx0 on ACT (first), hx1..3 on the
            # otherwise-idle Pool, hy-m0 on DVE right away; the hy-m1/m2
            # and h3s copies are interleaved into the first oct's stream
            hx = [
                cpool.tile([MSZ[m], N], fp16, name=f"hx{m}")
                for m in range(KT)
            ]
            hyball = cpool.tile([128, 192], f32, name="hyball")
            h3s = cpool.tile([128, 8], f32, name="h3s")
            # GPSIMD cannot read PSUM, so hx copies split over ACT/DVE
            if K_HXSPLIT == "a":
                nc.vector.tensor_copy(out=hx[0], in_=ph[0][:, :N])
                for m in (1, 2, 3):
                    nc.scalar.activation(
                        out=hx[m], in_=ph[m][:, :N], func=AF.Copy, bias=0.0,
                        scale=1.0,
                    )
            else:
                for m in (0, 1):
                    nc.scalar.activation(
                        out=hx[m], in_=ph[m][:, :N], func=AF.Copy, bias=0.0,
                        scale=1.0,
                    )
            nc.vector.tensor_copy(
                out=hyball[:, 0:NL], in_=pyh[:, 0:NL],
            )

            def late_copies(step):
                # emitted between the first oct's k-blocks (DVE stream)
                if step == 0:
                    nc.vector.tensor_copy(
                        out=hyball[:, NL : 2 * NL], in_=pyh[:, NL : 2 * NL],
                    )
                    if K_HXSPLIT == "b":
                        nc.vector.tensor_copy(out=hx[2], in_=ph[2][:, :N])
                elif step == 1:
                    nc.vector.tensor_copy(
                        out=hyball[:, 2 * NL : 3 * NL],
                        in_=pyh[:, 2 * NL : 3 * NL],
                    )
                    if K_HXSPLIT == "b":
                        nc.vector.tensor_copy(out=hx[3], in_=ph[3][:, :N])
                else:
                    nc.vector.tensor_copy(
                        out=h3s[0:64, :], in_=pyh[0:64, 192:200]
                    )
                    nc.vector.tensor_copy(
                        out=h3s[64:128, :], in_=pyh[0:64, 200:208]
                    )

            def hyb(m, i, direct=False):
                # per-partition scalar for H-tile m, row i; direct=True
                # reads straight from PSUM (first oct: skips waiting for
                # the hyball copy)
                src = pyh if direct else hyball
                return src[: MSZ[m], m * NL + i : m * NL + i + 1]

            def h3col(t):
                return h3s[:, t : t + 1]

            # ---- main loop: two 32-row halves, octs of 8 rows ----
            pso = [[pbank(f"ps{h}{jb}") for jb in range(4)] for h in range(2)]

            def relu_tile(eng, out, in_, scalar):
                if eng == "D":
                    nc.vector.tensor_scalar(
                        out=out, in0=in_, scalar1=scalar, scalar2=0.0,
                        op0=ALU.add, op1=ALU.max,
                    )
                elif eng == "P":
                    nc.gpsimd.tensor_scalar(
                        out=out, in0=in_, scalar1=scalar, scalar2=0.0,
                        op0=ALU.add, op1=ALU.max,
                    )
                else:
                    nc.scalar.activation(
                        out=out, in_=in_, func=AF.Relu, bias=scalar,
                        scale=1.0,
                    )

            si = [0]  # schedule cursor

            def next_eng():
                e = sched[si[0] % len(sched)]
                si[0] += 1
                return e

            def emit_oct(half, o):
                # packed k3 tail tile first: its matmuls close the oct, so
                # producing it early keeps the in-order PE from stalling.
                # (oct 0 must wait for the woven-in h3s copies, so its rq
                # is emitted after the row tiles instead.)
                rq = None
                if K_RQ_EARLY and not (half == 0 and o == 0):
                    rq = rqpool.tile([128, N], fp16, name="rq", tag="rq")
                    relu_tile(next_eng(), rq, hx[3], h3col(4 * half + o))
                for k in range(3):
                    for r in range(8):
                        rr = 8 * o + r
                        i = half * NH + rr
                        rt = wpool.tile([128, N], fp16, name="rt", tag="rt")
                        eng = next_eng()
                        direct = (K_DIRECT and half == 0 and o == 0
                                  and eng == "D")
                        relu_tile(eng, rt, hx[k], hyb(k, i, direct))
                        for jb in range(4):
                            nc.tensor.matmul(
                                pso[half][jb][:, :NH],
                                lhsT=rt[:, 128 * jb : 128 * (jb + 1)],
                                rhs=bshl[k][:, 64 - rr : 96 - rr],
                                start=(o == 0 and k == 0 and r == 0),
                                stop=False,
                            )
                        if half == 0 and o == 0 and r == 0:
                            late_copies(k)
                if rq is None:
                    rq = rqpool.tile([128, N], fp16, name="rq", tag="rq")
                    relu_tile(next_eng(), rq, hx[3], h3col(4 * half + o))
                for jb in range(4):
                    nc.tensor.matmul(
                        pso[half][jb][:, :NH],
                        lhsT=rq[:, 128 * jb : 128 * (jb + 1)],
                        rhs=b3t[:, 24 - 8 * o : 56 - 8 * o],
                        start=False, stop=(o == NH // 8 - 1),
                    )

            epi = {}

            def emit_epilogue_a(half):
                # ACT + PE part: exp/ln tiles + row-sum matmuls
                e2 = epool.tile([128, 128], fp16, name="e2", tag="e2")
                t1 = epool.tile([128, 128], fp16, name="t1", tag="t1")
                # E^T blocks = exp(s + b2), [128j, 32i] per j-block
                for jb in range(4):
                    nc.scalar.activation(
                        out=e2[:, 32 * jb : 32 * (jb + 1)],
                        in_=pso[half][jb][:, :NH],
                        func=AF.Exp, bias=b2t, scale=1.0,
                    )
                # T1^T = log(1 + E^T)
                nc.scalar.activation(
                    out=t1, in_=e2, func=AF.Ln, bias=onef, scale=1.0,
                )
                # row sums over j via ones-vector matmuls (free size 1)
                po = pbank(f"po{half}")
                for jb in range(4):
                    nc.tensor.matmul(
                        po[:NH, 0:1], lhsT=e2[:, 32 * jb : 32 * (jb + 1)],
                        rhs=one16, start=(jb == 0), stop=(jb == 3),
                    )
                for jb in range(4):
                    nc.tensor.matmul(
                        po[:NH, 1:2], lhsT=t1[:, 32 * jb : 32 * (jb + 1)],
                        rhs=one16, start=(jb == 0), stop=(jb == 3),
                    )
                epi[half] = (e2, po)

            def emit_epilogue_b(half):
                # DVE + DMA part: diag, out3 assembly, output DMA
                e2, po = epi[half]
                osl = slice(half * NH, (half + 1) * NH)
                tmp = epool.tile([NL, NH], fp16, name="tmp", tag="tmp")
                # ed = diag(E): rotated x puts the diag block at
                # j in [32h, 32h+32), i.e. partitions 32h.. of j-block 0
                nc.vector.tensor_tensor(
                    out=tmp[osl, :],
                    in0=e2[half * NH : (half + 1) * NH, 0:NH],
                    in1=maskh[half],
                    op=ALU.mult,
                )
                nc.vector.reduce_sum(
                    out=out3[osl, 2:3], in_=tmp[osl, :],
                    axis=mybir.AxisListType.X,
                )
                nc.vector.tensor_copy(out=out3[osl, 0:2], in_=po[:NH, 0:2])
                nc.sync.dma_start(out=out_o[osl, :], in_=out3[osl, :])

            for o in range(NH // 8):
                emit_oct(0, o)
            for o in range(NH // 8):
                emit_oct(1, o)
                if o == 0:
                    emit_epilogue_a(0)
                if o == 2:
                    emit_epilogue_b(0)
            emit_epilogue_a(1)
            emit_epilogue_b(1)

    nc.compile()
    return nc


def _make_in_maps(x, y, W1, b1, W2):
    f16 = np.float16
    slabA = np.zeros((401, ATOT), f16)
    slabB = np.zeros((401, BTOT), f16)
    w1xT = W1[:, :D].T.astype(f16)       # [D(k), H(m)]
    w1yT = W1[:, D:].T.astype(f16)
    slabA[:D, CW1X : CW1X + 384] = w1xT[:, :384]
    slabA[:D, CW1X + 384 : CW1X + 512] = np.tile(w1xT[:, 384:400], (1, 8))
    slabB[:D, CW1Y : CW1Y + 384] = w1yT[:, :384]
    slabB[400, CW1Y : CW1Y + 384] = b1[:384].astype(f16)
    # parity blocks: [384:416) = [w1y3 | 0], [400:432) = [0 | w1y3]
    slabB[:D, CW1Y + 384 : CW1Y + 400] = w1yT[:, 384:400]
    slabB[400, CW1Y + 384 : CW1Y + 400] = b1[384:400].astype(f16)
    slabB[:D, CW1Y + 416 : CW1Y + 432] = w1yT[:, 384:400]
    slabB[400, CW1Y + 416 : CW1Y + 432] = b1[384:400].astype(f16)

    consts = np.zeros((128, CTOT), f16)
    for k in range(3):
        consts[:128, CBSH + 128 * k + 64] = W2[0, 128 * k : 128 * (k + 1)].astype(f16)
    p = np.arange(128)
    consts[p, CB3 + 24 + 2 * (p // 32) + (p % 32) // 16] = W2[0, 384 + (p % 16)].astype(f16)
    consts[:NH, CMSK : CMSK + NH] = np.eye(NH, dtype=f16)
    consts[NH : 2 * NH, CMSK : CMSK + NH] = np.eye(NH, dtype=f16)

    xT = x.T.astype(f16)
    in_maps = []
    for c in range(NCORES):
        sa = slabA.copy()
        # rotate x columns so core c's diag block lands at columns [0, 64)
        sa[:D, CX : CX + N] = np.roll(xT, -c * NL, axis=1)
        sb = slabB.copy()
        sb[:D, CYT : CYT + NL] = y[c * NL : (c + 1) * NL, :].T.astype(f16)
        sb[400, CYT : CYT + NL] = 1.0
        in_maps.append({"slabA": sa, "slabB": sb, "constd": consts})
    return in_maps


def _combine(results):
    rr = np.concatenate([r["out_o"][:, 0].astype(np.float64) for r in results])
    rs = np.concatenate([r["out_o"][:, 1].astype(np.float64) for r in results])
    ed = np.concatenate([r["out_o"][:, 2].astype(np.float64) for r in results])
    lse = np.log(np.float64(N) + rr)
    t0 = np.log1p(ed)
    t0_mean = t0.mean()
    lower = t0_mean - (lse.mean() - np.log(np.float64(N)))
    upper = t0_mean - rs.mean() / N
    return np.float32(lower), np.float32(upper)


def kernel(x_samples, y_samples, W1, b1, W2, b2, _trace=False):
    from concourse.bass_utils import run_bass_kernel_spmd

    nc = _build_program(float(np.float32(b2[0])))
    in_maps = _make_in_maps(
        np.asarray(x_samples, np.float32),
        np.asarray(y_samples, np.float32),
        np.asarray(W1, np.float32),
        np.asarray(b1, np.float32),
        np.asarray(W2, np.float32),
    )
    res = run_bass_kernel_spmd(
        nc, in_maps, core_ids=list(range(NCORES)), trace=_trace
    )
    out = _combine(res.results)
    if _trace:
        return out, res
    return out


# revision 55
# speedup vs baseline: 1.4260x; 1.0085x over previous
"""CLUB-NCE loss kernel for 8x Trainium2 NeuronCores (Bass/Tile).

Math (reference):
  hx = x @ W1x.T, hy = y @ W1y.T            [N, H]
  s[i,j]  = W2 . relu(hy[i] + hx[j] + b1) + b2
  T1[i,j] = softplus(s[i,j]); T0[i] = T1[i,i]
  lower = mean(T0) - (mean_i(logsumexp_j(T1[i,:])) - log N)
  upper = mean(T0) - mean(T1)

Sharding: y rows (i axis) split across 8 cores (64 rows each); x and MLP
params replicated. Each core computes its [64, 512] score block and emits
per-row partials (row sum of e^s, row sum of T1, diag e^s). Host combines.

Device design (v2 — transposed score matmuls):
 - relu tiles r[k](i) = relu(hx[k] + hy[i]) [128k, 512j] are produced on
   DVE/ACT/Pool (greedy load-balanced), then used as the matmul
   STATIONARY: matmul(out=[128j, 32i], lhsT=r[:, jb*128:...],
   rhs=bsh[k][:, 64-rr : 96-rr]) routes W2.r into output column rr.
   Output free size is 32, so each matmul is tiny; 4 j-blocks x 3 k-tiles
   x 32 rows + 4 packed-tail matmuls per half.
 - the 16-deep k3 tails of 8 rows are packed into one [128, 512] relu
   tile (hx tail oct-duplicated on partitions, h3 scalars packed
   16r+m -> hy3[m, row 8t+r]) and contracted by one matmul per j-block
   with a banded stationary b3t[p, 24 + p//16] = w2[384 + p%16].
 - scores live transposed [j, i]; per-half epilogue: ACT exp/ln into
   [128, 4*32] SBUF tiles, then ones-vector matmuls (output free size 1)
   produce row sums over j on PSUM partitions; diag via identity mask
   (x columns rotated per core so the diag block is at j in [0, 64)).
 - b1 folded into the hy matmul as a virtual k=400 row.
 - prologue is k-batch ordered (one batch per arriving input slab);
   warmup dummies anchor the PE p-state ramp.
 - A slabs (x|w1x) on the SP HWDGE queue; B slabs (w1y|w1y3|yt) and the
   merged consts tile (bsh|b3t|mask) on the Pool SWDGE queue.
 - host finishes: lse_i = log(N + rr_i), t0_i = log(1 + ed_i), means.
"""

import numpy as np

N = 512          # number of samples
D = 400          # feature dim
H = 400          # hidden dim
NCORES = 8
NL = N // NCORES  # 64 y-rows per core
NH = NL // 2      # 32 rows per half
KT = 4            # k tiles
KSZ = [128, 128, 128, 16]    # real k per tile (400 total)
KSZY = [128, 128, 128, 17]   # hy matmul k per tile (incl. bias row)
# consolidated input slabs:
#   A: x | w1x m0-2 | w1x m3-oct (8 copies of the 16 tail columns)
#   B: w1y m0-2 | w1y3 parity blocks | yt
CX, CW1X = 0, 512
ATOT = 1024
CW1Y, CYT = 0, 432
BTOT = 496
# consts tile: bsh (3x128) | b3t (56) | mask (32, at partitions 0:32)
CBSH, CB3, CMSK = 0, 384, 440
CTOT = 472
MSZ = [128, 128, 128, 128]   # H-tile partition sizes (m3 oct-duplicated)
NWARM = 1         # PE warmup dummy matmuls (anchors the p-state ramp)
DROWS = 384       # dummy matmul free size

import os as _os

# relu-tile engine split: greedy balance by per-tile engine cost (ns,
# measured in TimelineSim: free-size*cycle + init/2 [+ Q7 launch])
ENG_COST = {"D": 194.0, "A": 630.0, "P": 806.0}
# initial load offsets (ns): ACT pays the mid-window epilogue-A AND the
# terminal exp/ln chain (its stream must end first); Pool starts late
# (SWDGE prologue + hx copies)
ENG_SEED = {
    "D": float(_os.environ.get("K_SEED_D", 0.0)),
    "A": float(_os.environ.get("K_SEED_A", 1000.0)),
    "P": float(_os.environ.get("K_SEED_P", 200.0)),
}
K_RQ_EARLY = int(_os.environ.get("K_RQ_EARLY", 1))
K_DIRECT = int(_os.environ.get("K_DIRECT", 0))
K_HXSPLIT = _os.environ.get("K_HXSPLIT", "a")  # a: hx0 DVE + hx123 ACT
                                               # b: hx01 ACT + hx23 DVE-woven


def _relu_schedule():
    """Greedy assignment of the 200 relu tiles to engines."""
    load = dict(ENG_SEED)
    out = []
    for _ in range(2 * 4 * 25):  # halves x octs x (24 row tiles + rq)
        e = min(load, key=lambda x: load[x] + ENG_COST[x])
        load[e] += ENG_COST[e]
        out.append(e)
    return out


def _build_program(b2val: float, enable_asserts: bool = False):
    import concourse.bacc as bacc
    import concourse.mybir as mybir
    import concourse.tile as tile

    fp16 = mybir.dt.float16
    f32 = mybir.dt.float32
    AF = mybir.ActivationFunctionType
    ALU = mybir.AluOpType

    nc = bacc.Bacc(
        "TRN2",
        target_bir_lowering=False,
        debug=False,
        enable_asserts=enable_asserts,
    )

    slabA = nc.dram_tensor("slabA", [401, ATOT], fp16, kind="ExternalInput")
    slabB = nc.dram_tensor("slabB", [401, BTOT], fp16, kind="ExternalInput")
    constd = nc.dram_tensor("constd", [128, CTOT], fp16, kind="ExternalInput")
    out_o = nc.dram_tensor("out_o", [NL, 4], f32, kind="ExternalOutput")

    sched = _relu_schedule()

    with tile.TileContext(nc) as tc:
        with (
            tc.tile_pool(name="const", bufs=1) as cpool,
            tc.tile_pool(name="work", bufs=48) as wpool,
            tc.tile_pool(name="rq", bufs=4) as rqpool,
            tc.tile_pool(name="epi", bufs=2) as epool,
            tc.tile_pool(name="psum", bufs=8, space="PSUM") as pp,
        ):
            # one table load (natural_log_exp_and_others: copy/relu/exp/ln)
            nc.scalar.add_instruction(
                mybir.InstLoadActFuncSet(
                    name=nc.get_next_instruction_name(),
                    act_func_set_id=6,
                    engine=mybir.EngineType.Activation,
                    ins=[],
                    outs=[],
                )
            )

            # ---- input DMAs: A pieces on SP (feed hx), B + consts on Pool
            sa_t, sb_t = [], []
            for k in range(KT):
                t = cpool.tile([KSZ[k], ATOT], fp16, name=f"slabA{k}")
                nc.sync.dma_start(
                    out=t, in_=slabA[k * 128 : k * 128 + KSZ[k], :]
                )
                sa_t.append(t)
            for k in range(KT):
                t = cpool.tile([KSZY[k], BTOT], fp16, name=f"slabB{k}")
                # B0/B1 on Pool SWDGE (parallel issue channel), B2/B3 on
                # the SP queue behind the A slabs: Pool's engine frees by
                # ~3us and the last B piece still lands earlier
                q = nc.gpsimd if k < 3 else nc.sync
                q.dma_start(
                    out=t, in_=slabB[k * 128 : k * 128 + KSZY[k], :]
                )
                sb_t.append(t)
            # consts ride the SP queue after the A slabs (keeps Pool's
            # SWDGE queue short so the B slabs land earlier)
            cons = cpool.tile([128, CTOT], fp16, name="cons")
            nc.sync.dma_start(out=cons, in_=constd[:, :])

            xt = [sa_t[k][:, CX : CX + N] for k in range(KT)]
            w1x = [sa_t[k][:, CW1X : CW1X + 512] for k in range(KT)]
            w1y = [sb_t[k][:, CW1Y : CW1Y + 384] for k in range(KT)]
            w1y3e = [sb_t[k][:, CW1Y + 384 : CW1Y + 416] for k in range(KT)]
            w1y3o = [sb_t[k][:, CW1Y + 400 : CW1Y + 432] for k in range(KT)]
            yt = [sb_t[k][:, CYT : CYT + NL] for k in range(KT)]
            bshl = [
                cons[: KSZ[k], CBSH + 128 * k : CBSH + 128 * (k + 1)]
                for k in range(3)
            ]
            b3t = cons[:, CB3 : CB3 + 56]
            # two stacked identity copies so each half's diag extraction
            # reads a mask at its own base partition (32h)
            maskh = [cons[32 * h : 32 * h + NH, CMSK : CMSK + NH]
                     for h in range(2)]

            # ---- small consts ----
            dumw = cpool.tile([128, 1], fp16, name="dumw")
            nc.vector.memset(dumw, 0.0)
            dumr = cpool.tile([128, DROWS], fp16, name="dumr")
            nc.vector.memset(dumr, 0.0)
            b2t = cpool.tile([128, 1], f32, name="b2t")
            nc.vector.memset(b2t, b2val)
            onef = cpool.tile([128, 1], f32, name="onef")
            nc.vector.memset(onef, 1.0)
            one16 = cpool.tile([128, 1], fp16, name="one16")
            nc.vector.memset(one16, 1.0)
            out3 = cpool.tile([NL, 4], f32, name="out3")
            nc.vector.memset(out3, 0.0)

            # ---- prologue PSUM pool (closed before the main loop so its
            # 6 banks are re-used by the two quad score tiles)
            ppro_cm = tc.tile_pool(name="pspro", bufs=1, space="PSUM")
            ppro = ppro_cm.__enter__()
            pd = ppro.tile([128, 512], f32, name="pd", tag="pd")
            ph = [ppro.tile([128, 512], f32, name=f"ph{m}", tag=f"ph{m}")
                  for m in range(KT)]
            pyh = ppro.tile([128, 512], f32, name="pyh", tag="pyh")
            # pyh cols 0:192 hy m-blocks, 192:208 h3

            def dummies(n, free=DROWS):
                for _ in range(n):
                    nc.tensor.matmul(pd[:1, :free], lhsT=dumw, rhs=dumr[:, :free],
                                     start=True, stop=True)

            dummies(NWARM)

            def hx_batch(k, ms):
                for m in ms:
                    msl = (slice(m * 128, (m + 1) * 128) if m < 3
                           else slice(384, 512))
                    nc.tensor.matmul(
                        ph[m][:, :N], lhsT=w1x[k][:, msl], rhs=xt[k],
                        start=(k == 0), stop=(k == KT - 1),
                    )

            # k0..k2 batches in full; then ONLY m0 of the k3 batch so the
            # hx0 copy (and with it the first relu tiles) unblocks before
            # the rest of the prologue drains
            for k in range(3):
                hx_batch(k, range(KT))
            hx_batch(3, [0])
            # hy blocks + h3 share one PSUM bank: groups sequential
            for m in range(3):
                msl = slice(m * 128, (m + 1) * 128)
                for k in range(KT):
                    nc.tensor.matmul(
                        pyh[:, m * NL : (m + 1) * NL],
                        lhsT=w1y[k][:, msl], rhs=yt[k],
                        start=(k == 0), stop=(k == KT - 1),
                    )
            # h3: per-partition-paired hy tail, packed like the baseline:
            # h3[32a+16p+m, t] = hy3[m, y-row 8t+2a+p] + b1[384+m].
            # h3lo (a=0,1) at pyh cols 192:200, h3hi (a=2,3) at 200:208.
            for a in range(4):
                csl = slice(192, 200) if a < 2 else slice(200, 208)
                psl = slice(32 * (a % 2), 32 * (a % 2) + 32)
                for par in range(2):
                    lh = w1y3e if par == 0 else w1y3o
                    for k in range(KT):
                        nc.tensor.matmul(
                            pyh[psl, csl],
                            lhsT=lh[k],
                            rhs=yt[k][:, 2 * a + par : NL : 8],
                            start=(par == 0 and k == 0),
                            stop=(par == 1 and k == KT - 1),
                        )
            # finish the hx k3 batch after hy/h3 so hx0 unblocked early
            hx_batch(3, [1, 2, 3])

            # psum -> sbuf staging: hx0 on ACT (first), hx1..3 on the
            # otherwise-idle Pool, hy-m0 on DVE right away; the hy-m1/m2
            # and h3s copies are interleaved into the first oct's stream
            hx = [
                cpool.tile([MSZ[m], N], fp16, name=f"hx{m}")
                for m in range(KT)
            ]
            hyball = cpool.tile([128, 192], f32, name="hyball")
            h3s = cpool.tile([128, 8], f32, name="h3s")
            # GPSIMD cannot read PSUM, so hx copies split over ACT/DVE;
            # all copies run pre-main (the prologue pool closes below)
            nc.vector.tensor_copy(out=hx[0], in_=ph[0][:, :N])
            nc.vector.tensor_copy(
                out=hyball[:, 0:NL], in_=pyh[:, 0:NL],
            )
            for m in (1, 2, 3):
                nc.scalar.activation(
                    out=hx[m], in_=ph[m][:, :N], func=AF.Copy, bias=0.0,
                    scale=1.0,
                )
            for m in (1, 2):
                nc.vector.tensor_copy(
                    out=hyball[:, m * NL : (m + 1) * NL],
                    in_=pyh[:, m * NL : (m + 1) * NL],
                )
            nc.vector.tensor_copy(out=h3s[0:64, :], in_=pyh[0:64, 192:200])
            nc.vector.tensor_copy(out=h3s[64:128, :], in_=pyh[0:64, 200:208])

            ppro_cm.__exit__(None, None, None)

            def hyb(m, i):  # per-partition scalar for H-tile m, row i
                return hyball[: MSZ[m], m * NL + i : m * NL + i + 1]

            def h3col(t):
                return h3s[:, t : t + 1]

            # ---- main loop: two 32-row halves, octs of 8 rows ----
            # each half's scores live in ONE [128, 2048] quad-bank tile:
            # j-block jb's [128, 32] block at cols 512*jb (bank-aligned,
            # so the 4 accumulation groups stay on separate banks), and
            # the half's row-sum columns at cols 1568:1570 (sequential
            # groups on bank 3 after jb3 stops)
            psq_cm = tc.tile_pool(name="psq", bufs=1, space="PSUM")
            psq = psq_cm.__enter__()
            qso = [psq.tile([128, 2048], f32, name=f"q{h}", tag=f"q{h}")
                   for h in range(2)]

            def pso_ap(half, jb):
                return qso[half][:, 512 * jb : 512 * jb + NH]

            def relu_tile(eng, out, in_, scalar):
                if eng == "D":
                    nc.vector.tensor_scalar(
                        out=out, in0=in_, scalar1=scalar, scalar2=0.0,
                        op0=ALU.add, op1=ALU.max,
                    )
                elif eng == "P":
                    nc.gpsimd.tensor_scalar(
                        out=out, in0=in_, scalar1=scalar, scalar2=0.0,
                        op0=ALU.add, op1=ALU.max,
                    )
                else:
                    nc.scalar.activation(
                        out=out, in_=in_, func=AF.Relu, bias=scalar,
                        scale=1.0,
                    )

            si = [0]  # schedule cursor

            def next_eng():
                e = sched[si[0] % len(sched)]
                si[0] += 1
                return e

            def emit_oct(half, o):
                # packed k3 tail tile first: its matmuls close the oct, so
                # producing it early keeps the in-order PE from stalling
                rq = None
                if K_RQ_EARLY:
                    rq = rqpool.tile([128, N], fp16, name="rq", tag="rq")
                    relu_tile(next_eng(), rq, hx[3], h3col(4 * half + o))
                for k in range(3):
                    for r in range(8):
                        rr = 8 * o + r
                        i = half * NH + rr
                        rt = wpool.tile([128, N], fp16, name="rt", tag="rt")
                        relu_tile(next_eng(), rt, hx[k], hyb(k, i))
                        for jb in range(4):
                            nc.tensor.matmul(
                                pso_ap(half, jb),
                                lhsT=rt[:, 128 * jb : 128 * (jb + 1)],
                                rhs=bshl[k][:, 64 - rr : 96 - rr],
                                start=(o == 0 and k == 0 and r == 0),
                                stop=False,
                            )
                if rq is None:
                    rq = rqpool.tile([128, N], fp16, name="rq", tag="rq")
                    relu_tile(next_eng(), rq, hx[3], h3col(4 * half + o))
                for jb in range(4):
                    nc.tensor.matmul(
                        pso_ap(half, jb),
                        lhsT=rq[:, 128 * jb : 128 * (jb + 1)],
                        rhs=b3t[:, 24 - 8 * o : 56 - 8 * o],
                        start=False, stop=(o == NH // 8 - 1),
                    )

            epi = {}

            def emit_epilogue_a(half):
                # ACT + PE part: exp/ln tiles + row-sum matmuls
                e2 = epool.tile([128, 128], fp16, name="e2", tag="e2")
                t1 = epool.tile([128, 128], fp16, name="t1", tag="t1")
                # E^T = exp(s + b2): ONE activation over all 4 j-blocks
                # via a strided AP on the quad tile ([bank, 4] middle dim)
                qap = qso[half][:, 0:2048]
                e_in = type(qap)(
                    qap.tensor, qap.offset,
                    [list(qap.ap[0]), [512, 4], [1, NH]],
                )
                nc.scalar.activation(
                    out=e2, in_=e_in, func=AF.Exp, bias=b2t, scale=1.0,
                )
                # T1^T = log(1 + E^T)
                nc.scalar.activation(
                    out=t1, in_=e2, func=AF.Ln, bias=onef, scale=1.0,
                )
                # row sums over j via ones-vector matmuls (free size 1);
                # accumulate into bank 3 of the quad (after jb3's cols,
                # groups sequential per bank)
                po = qso[half][:NH, 1568:1570]
                for jb in range(4):
                    nc.tensor.matmul(
                        po[:, 0:1], lhsT=e2[:, 32 * jb : 32 * (jb + 1)],
                        rhs=one16, start=(jb == 0), stop=(jb == 3),
                    )
                for jb in range(4):
                    nc.tensor.matmul(
                        po[:, 1:2], lhsT=t1[:, 32 * jb : 32 * (jb + 1)],
                        rhs=one16, start=(jb == 0), stop=(jb == 3),
                    )
                epi[half] = (e2, po)

            def emit_epilogue_b(half):
                # DVE + DMA part: diag, out3 assembly, output DMA
                e2, po = epi[half]
                osl = slice(half * NH, (half + 1) * NH)
                tmp = epool.tile([NL, NH], fp16, name="tmp", tag="tmp")
                # ed = diag(E): rotated x puts the diag block at
                # j in [32h, 32h+32), i.e. partitions 32h.. of j-block 0
                nc.vector.tensor_tensor(
                    out=tmp[osl, :],
                    in0=e2[half * NH : (half + 1) * NH, 0:NH],
                    in1=maskh[half],
                    op=ALU.mult,
                )
                nc.vector.reduce_sum(
                    out=out3[osl, 2:3], in_=tmp[osl, :],
                    axis=mybir.AxisListType.X,
                )
                nc.vector.tensor_copy(out=out3[osl, 0:2], in_=po[:, 0:2])
                nc.sync.dma_start(out=out_o[osl, :], in_=out3[osl, :])

            for o in range(NH // 8):
                emit_oct(0, o)
            for o in range(NH // 8):
                emit_oct(1, o)
                if o == 0:
                    emit_epilogue_a(0)
                if o == 2:
                    emit_epilogue_b(0)
            emit_epilogue_a(1)
            emit_epilogue_b(1)
            psq_cm.__exit__(None, None, None)

    nc.compile()
    return nc


def _make_in_maps(x, y, W1, b1, W2):
    f16 = np.float16
    slabA = np.zeros((401, ATOT), f16)
    slabB = np.zeros((401, BTOT), f16)
    w1xT = W1[:, :D].T.astype(f16)       # [D(k), H(m)]
    w1yT = W1[:, D:].T.astype(f16)
    slabA[:D, CW1X : CW1X + 384] = w1xT[:, :384]
    slabA[:D, CW1X + 384 : CW1X + 512] = np.tile(w1xT[:, 384:400], (1, 8))
    slabB[:D, CW1Y : CW1Y + 384] = w1yT[:, :384]
    slabB[400, CW1Y : CW1Y + 384] = b1[:384].astype(f16)
    # parity blocks: [384:416) = [w1y3 | 0], [400:432) = [0 | w1y3]
    slabB[:D, CW1Y + 384 : CW1Y + 400] = w1yT[:, 384:400]
    slabB[400, CW1Y + 384 : CW1Y + 400] = b1[384:400].astype(f16)
    slabB[:D, CW1Y + 416 : CW1Y + 432] = w1yT[:, 384:400]
    slabB[400, CW1Y + 416 : CW1Y + 432] = b1[384:400].astype(f16)

    consts = np.zeros((128, CTOT), f16)
    for k in range(3):
        consts[:128, CBSH + 128 * k + 64] = W2[0, 128 * k : 128 * (k + 1)].astype(f16)
    p = np.arange(128)
    consts[p, CB3 + 24 + 2 * (p // 32) + (p % 32) // 16] = W2[0, 384 + (p % 16)].astype(f16)
    consts[:NH, CMSK : CMSK + NH] = np.eye(NH, dtype=f16)
    consts[NH : 2 * NH, CMSK : CMSK + NH] = np.eye(NH, dtype=f16)

    xT = x.T.astype(f16)
    in_maps = []
    for c in range(NCORES):
        sa = slabA.copy()
        # rotate x columns so core c's diag block lands at columns [0, 64)
        sa[:D, CX : CX + N] = np.roll(xT, -c * NL, axis=1)
        sb = slabB.copy()
        sb[:D, CYT : CYT + NL] = y[c * NL : (c + 1) * NL, :].T.astype(f16)
        sb[400, CYT : CYT + NL] = 1.0
        in_maps.append({"slabA": sa, "slabB": sb, "constd": consts})
    return in_maps


def _combine(results):
    rr = np.concatenate([r["out_o"][:, 0].astype(np.float64) for r in results])
    rs = np.concatenate([r["out_o"][:, 1].astype(np.float64) for r in results])
    ed = np.concatenate([r["out_o"][:, 2].astype(np.float64) for r in results])
    lse = np.log(np.float64(N) + rr)
    t0 = np.log1p(ed)
    t0_mean = t0.mean()
    lower = t0_mean - (lse.mean() - np.log(np.float64(N)))
    upper = t0_mean - rs.mean() / N
    return np.float32(lower), np.float32(upper)


def kernel(x_samples, y_samples, W1, b1, W2, b2, _trace=False):
    from concourse.bass_utils import run_bass_kernel_spmd

    nc = _build_program(float(np.float32(b2[0])))
    in_maps = _make_in_maps(
        np.asarray(x_samples, np.float32),
        np.asarray(y_samples, np.float32),
        np.asarray(W1, np.float32),
        np.asarray(b1, np.float32),
        np.asarray(W2, np.float32),
    )
    res = run_bass_kernel_spmd(
        nc, in_maps, core_ids=list(range(NCORES)), trace=_trace
    )
    out = _combine(res.results)
    if _trace:
        return out, res
    return out


# revision 56
# speedup vs baseline: 1.4366x; 1.0074x over previous
"""CLUB-NCE loss kernel for 8x Trainium2 NeuronCores (Bass/Tile).

Math (reference):
  hx = x @ W1x.T, hy = y @ W1y.T            [N, H]
  s[i,j]  = W2 . relu(hy[i] + hx[j] + b1) + b2
  T1[i,j] = softplus(s[i,j]); T0[i] = T1[i,i]
  lower = mean(T0) - (mean_i(logsumexp_j(T1[i,:])) - log N)
  upper = mean(T0) - mean(T1)

Sharding: y rows (i axis) split across 8 cores (64 rows each); x and MLP
params replicated. Each core computes its [64, 512] score block and emits
per-row partials (row sum of e^s, row sum of T1, diag e^s). Host combines.

Device design (v2 — transposed score matmuls):
 - relu tiles r[k](i) = relu(hx[k] + hy[i]) [128k, 512j] are produced on
   DVE/ACT/Pool (greedy load-balanced), then used as the matmul
   STATIONARY: matmul(out=[128j, 32i], lhsT=r[:, jb*128:...],
   rhs=bsh[k][:, 64-rr : 96-rr]) routes W2.r into output column rr.
   Output free size is 32, so each matmul is tiny; 4 j-blocks x 3 k-tiles
   x 32 rows + 4 packed-tail matmuls per half.
 - the 16-deep k3 tails of 8 rows are packed into one [128, 512] relu
   tile (hx tail oct-duplicated on partitions, h3 scalars packed
   16r+m -> hy3[m, row 8t+r]) and contracted by one matmul per j-block
   with a banded stationary b3t[p, 24 + p//16] = w2[384 + p%16].
 - scores live transposed [j, i]; per-half epilogue: ACT exp/ln into
   [128, 4*32] SBUF tiles, then ones-vector matmuls (output free size 1)
   produce row sums over j on PSUM partitions; diag via identity mask
   (x columns rotated per core so the diag block is at j in [0, 64)).
 - b1 folded into the hy matmul as a virtual k=400 row.
 - prologue is k-batch ordered (one batch per arriving input slab);
   warmup dummies anchor the PE p-state ramp.
 - A slabs (x|w1x) on the SP HWDGE queue; B slabs (w1y|w1y3|yt) and the
   merged consts tile (bsh|b3t|mask) on the Pool SWDGE queue.
 - host finishes: lse_i = log(N + rr_i), t0_i = log(1 + ed_i), means.
"""

import numpy as np

N = 512          # number of samples
D = 400          # feature dim
H = 400          # hidden dim
NCORES = 8
NL = N // NCORES  # 64 y-rows per core
NH = NL // 2      # 32 rows per half
KT = 4            # k tiles
KSZ = [128, 128, 128, 16]    # real k per tile (400 total)
KSZY = [128, 128, 128, 17]   # hy matmul k per tile (incl. bias row)
# consolidated input slabs:
#   A: x | w1x m0-2 | w1x m3-oct (8 copies of the 16 tail columns)
#   B: w1y m0-2 | w1y3 parity blocks | yt
CX, CW1X = 0, 512
ATOT = 1024
CW1Y, CYT = 0, 432
BTOT = 496
# consts tile: bsh (3x128) | b3t (56) | mask (32, at partitions 0:32)
CBSH, CB3, CMSK = 0, 384, 440
CTOT = 472
MSZ = [128, 128, 128, 128]   # H-tile partition sizes (m3 oct-duplicated)
NWARM = 1         # PE warmup dummy matmuls (anchors the p-state ramp)
DROWS = 384       # dummy matmul free size

import os as _os

# relu-tile engine split: greedy balance by per-tile engine cost (ns,
# measured in TimelineSim: free-size*cycle + init/2 [+ Q7 launch])
ENG_COST = {"D": 194.0, "A": 630.0, "P": 806.0}
# initial load offsets (ns): ACT pays the mid-window epilogue-A AND the
# terminal exp/ln chain (its stream must end first); Pool starts late
# (SWDGE prologue + hx copies)
ENG_SEED = {
    "D": float(_os.environ.get("K_SEED_D", 0.0)),
    "A": float(_os.environ.get("K_SEED_A", 600.0)),
    "P": float(_os.environ.get("K_SEED_P", 200.0)),
}
K_RQ_EARLY = int(_os.environ.get("K_RQ_EARLY", 1))
K_DIRECT = int(_os.environ.get("K_DIRECT", 0))
K_HXSPLIT = _os.environ.get("K_HXSPLIT", "a")  # a: hx0 DVE + hx123 ACT
                                               # b: hx01 ACT + hx23 DVE-woven


def _relu_schedule():
    """Greedy assignment of the 200 relu tiles to engines."""
    load = dict(ENG_SEED)
    out = []
    for _ in range(2 * 4 * 25):  # halves x octs x (24 row tiles + rq)
        e = min(load, key=lambda x: load[x] + ENG_COST[x])
        load[e] += ENG_COST[e]
        out.append(e)
    return out


def _build_program(b2val: float, enable_asserts: bool = False):
    import concourse.bacc as bacc
    import concourse.mybir as mybir
    import concourse.tile as tile

    fp16 = mybir.dt.float16
    f32 = mybir.dt.float32
    AF = mybir.ActivationFunctionType
    ALU = mybir.AluOpType

    nc = bacc.Bacc(
        "TRN2",
        target_bir_lowering=False,
        debug=False,
        enable_asserts=enable_asserts,
    )

    slabA = nc.dram_tensor("slabA", [401, ATOT], fp16, kind="ExternalInput")
    slabB = nc.dram_tensor("slabB", [401, BTOT], fp16, kind="ExternalInput")
    constd = nc.dram_tensor("constd", [128, CTOT], fp16, kind="ExternalInput")
    out_o = nc.dram_tensor("out_o", [NL, 4], f32, kind="ExternalOutput")

    sched = _relu_schedule()

    with tile.TileContext(nc) as tc:
        with (
            tc.tile_pool(name="const", bufs=1) as cpool,
            tc.tile_pool(name="work", bufs=48) as wpool,
            tc.tile_pool(name="rq", bufs=4) as rqpool,
            tc.tile_pool(name="epi", bufs=2) as epool,
            tc.tile_pool(name="psum", bufs=8, space="PSUM") as pp,
        ):
            # one table load (natural_log_exp_and_others: copy/relu/exp/ln)
            nc.scalar.add_instruction(
                mybir.InstLoadActFuncSet(
                    name=nc.get_next_instruction_name(),
                    act_func_set_id=6,
                    engine=mybir.EngineType.Activation,
                    ins=[],
                    outs=[],
                )
            )

            # ---- input DMAs: A pieces on SP (feed hx), B + consts on Pool
            sa_t, sb_t = [], []
            for k in range(KT):
                t = cpool.tile([KSZ[k], ATOT], fp16, name=f"slabA{k}")
                nc.sync.dma_start(
                    out=t, in_=slabA[k * 128 : k * 128 + KSZ[k], :]
                )
                sa_t.append(t)
            for k in range(KT):
                t = cpool.tile([KSZY[k], BTOT], fp16, name=f"slabB{k}")
                # B0/B1 on Pool SWDGE (parallel issue channel), B2/B3 on
                # the SP queue behind the A slabs: Pool's engine frees by
                # ~3us and the last B piece still lands earlier
                q = nc.gpsimd if k < 3 else nc.sync
                q.dma_start(
                    out=t, in_=slabB[k * 128 : k * 128 + KSZY[k], :]
                )
                sb_t.append(t)
            # consts ride the SP queue after the A slabs (keeps Pool's
            # SWDGE queue short so the B slabs land earlier)
            cons = cpool.tile([128, CTOT], fp16, name="cons")
            nc.sync.dma_start(out=cons, in_=constd[:, :])

            xt = [sa_t[k][:, CX : CX + N] for k in range(KT)]
            w1x = [sa_t[k][:, CW1X : CW1X + 512] for k in range(KT)]
            w1y = [sb_t[k][:, CW1Y : CW1Y + 384] for k in range(KT)]
            w1y3e = [sb_t[k][:, CW1Y + 384 : CW1Y + 416] for k in range(KT)]
            w1y3o = [sb_t[k][:, CW1Y + 400 : CW1Y + 432] for k in range(KT)]
            yt = [sb_t[k][:, CYT : CYT + NL] for k in range(KT)]
            bshl = [
                cons[: KSZ[k], CBSH + 128 * k : CBSH + 128 * (k + 1)]
                for k in range(3)
            ]
            b3t = cons[:, CB3 : CB3 + 56]
            # two stacked identity copies so each half's diag extraction
            # reads a mask at its own base partition (32h)
            maskh = [cons[32 * h : 32 * h + NH, CMSK : CMSK + NH]
                     for h in range(2)]

            # ---- small consts ----
            dumw = cpool.tile([128, 1], fp16, name="dumw")
            nc.vector.memset(dumw, 0.0)
            dumr = cpool.tile([128, DROWS], fp16, name="dumr")
            nc.vector.memset(dumr, 0.0)
            b2t = cpool.tile([128, 1], f32, name="b2t")
            nc.vector.memset(b2t, b2val)
            onef = cpool.tile([128, 1], f32, name="onef")
            nc.vector.memset(onef, 1.0)
            one16 = cpool.tile([128, 1], fp16, name="one16")
            nc.vector.memset(one16, 1.0)
            out3 = cpool.tile([NL, 4], f32, name="out3")
            nc.vector.memset(out3, 0.0)

            # ---- prologue PSUM pool (closed before the main loop so its
            # 6 banks are re-used by the two quad score tiles)
            ppro_cm = tc.tile_pool(name="pspro", bufs=1, space="PSUM")
            ppro = ppro_cm.__enter__()
            pd = ppro.tile([128, 512], f32, name="pd", tag="pd")
            ph = [ppro.tile([128, 512], f32, name=f"ph{m}", tag=f"ph{m}")
                  for m in range(KT)]
            pyh = ppro.tile([128, 512], f32, name="pyh", tag="pyh")
            # pyh cols 0:192 hy m-blocks, 192:208 h3

            def dummies(n, free=DROWS):
                for _ in range(n):
                    nc.tensor.matmul(pd[:1, :free], lhsT=dumw, rhs=dumr[:, :free],
                                     start=True, stop=True)

            dummies(NWARM)

            def hx_batch(k, ms):
                for m in ms:
                    msl = (slice(m * 128, (m + 1) * 128) if m < 3
                           else slice(384, 512))
                    nc.tensor.matmul(
                        ph[m][:, :N], lhsT=w1x[k][:, msl], rhs=xt[k],
                        start=(k == 0), stop=(k == KT - 1),
                    )

            # k0..k2 batches in full; then ONLY m0 of the k3 batch so the
            # hx0 copy (and with it the first relu tiles) unblocks before
            # the rest of the prologue drains
            for k in range(3):
                hx_batch(k, range(KT))
            hx_batch(3, [0])
            # hy blocks + h3 share one PSUM bank: groups sequential
            for m in range(3):
                msl = slice(m * 128, (m + 1) * 128)
                for k in range(KT):
                    nc.tensor.matmul(
                        pyh[:, m * NL : (m + 1) * NL],
                        lhsT=w1y[k][:, msl], rhs=yt[k],
                        start=(k == 0), stop=(k == KT - 1),
                    )
            # h3: per-partition-paired hy tail, packed like the baseline:
            # h3[32a+16p+m, t] = hy3[m, y-row 8t+2a+p] + b1[384+m].
            # h3lo (a=0,1) at pyh cols 192:200, h3hi (a=2,3) at 200:208.
            for a in range(4):
                csl = slice(192, 200) if a < 2 else slice(200, 208)
                psl = slice(32 * (a % 2), 32 * (a % 2) + 32)
                for par in range(2):
                    lh = w1y3e if par == 0 else w1y3o
                    for k in range(KT):
                        nc.tensor.matmul(
                            pyh[psl, csl],
                            lhsT=lh[k],
                            rhs=yt[k][:, 2 * a + par : NL : 8],
                            start=(par == 0 and k == 0),
                            stop=(par == 1 and k == KT - 1),
                        )
            # finish the hx k3 batch after hy/h3 so hx0 unblocked early
            hx_batch(3, [1, 2, 3])

            # psum -> sbuf staging: hx0 on ACT (first), hx1..3 on the
            # otherwise-idle Pool, hy-m0 on DVE right away; the hy-m1/m2
            # and h3s copies are interleaved into the first oct's stream
            hx = [
                cpool.tile([MSZ[m], N], fp16, name=f"hx{m}")
                for m in range(KT)
            ]
            hyball = cpool.tile([128, 192], f32, name="hyball")
            h3s = cpool.tile([128, 8], f32, name="h3s")
            # GPSIMD cannot read PSUM, so hx copies split over ACT/DVE;
            # all copies run pre-main (the prologue pool closes below)
            nc.vector.tensor_copy(out=hx[0], in_=ph[0][:, :N])
            nc.vector.tensor_copy(
                out=hyball[:, 0:NL], in_=pyh[:, 0:NL],
            )
            for m in (1, 2, 3):
                nc.scalar.activation(
                    out=hx[m], in_=ph[m][:, :N], func=AF.Copy, bias=0.0,
                    scale=1.0,
                )
            for m in (1, 2):
                nc.vector.tensor_copy(
                    out=hyball[:, m * NL : (m + 1) * NL],
                    in_=pyh[:, m * NL : (m + 1) * NL],
                )
            nc.vector.tensor_copy(out=h3s[0:64, :], in_=pyh[0:64, 192:200])
            nc.vector.tensor_copy(out=h3s[64:128, :], in_=pyh[0:64, 200:208])

            ppro_cm.__exit__(None, None, None)

            def hyb(m, i):  # per-partition scalar for H-tile m, row i
                return hyball[: MSZ[m], m * NL + i : m * NL + i + 1]

            def h3col(t):
                return h3s[:, t : t + 1]

            # ---- main loop: two 32-row halves, octs of 8 rows ----
            # each half's scores live in ONE [128, 2048] quad-bank tile:
            # j-block jb's [128, 32] block at cols 512*jb (bank-aligned,
            # so the 4 accumulation groups stay on separate banks), and
            # the half's row-sum columns at cols 1568:1570 (sequential
            # groups on bank 3 after jb3 stops)
            psq_cm = tc.tile_pool(name="psq", bufs=1, space="PSUM")
            psq = psq_cm.__enter__()
            qso = [psq.tile([128, 2048], f32, name=f"q{h}", tag=f"q{h}")
                   for h in range(2)]

            def pso_ap(half, jb):
                return qso[half][:, 512 * jb : 512 * jb + NH]

            def relu_tile(eng, out, in_, scalar):
                if eng == "D":
                    nc.vector.tensor_scalar(
                        out=out, in0=in_, scalar1=scalar, scalar2=0.0,
                        op0=ALU.add, op1=ALU.max,
                    )
                elif eng == "P":
                    nc.gpsimd.tensor_scalar(
                        out=out, in0=in_, scalar1=scalar, scalar2=0.0,
                        op0=ALU.add, op1=ALU.max,
                    )
                else:
                    nc.scalar.activation(
                        out=out, in_=in_, func=AF.Relu, bias=scalar,
                        scale=1.0,
                    )

            si = [0]  # schedule cursor

            def next_eng():
                e = sched[si[0] % len(sched)]
                si[0] += 1
                return e

            def emit_oct(half, o):
                # packed k3 tail tile first: its matmuls close the oct, so
                # producing it early keeps the in-order PE from stalling
                rq = None
                if K_RQ_EARLY:
                    rq = rqpool.tile([128, N], fp16, name="rq", tag="rq")
                    relu_tile(next_eng(), rq, hx[3], h3col(4 * half + o))
                for k in range(3):
                    for r in range(8):
                        rr = 8 * o + r
                        i = half * NH + rr
                        rt = wpool.tile([128, N], fp16, name="rt", tag="rt")
                        relu_tile(next_eng(), rt, hx[k], hyb(k, i))
                        for jb in range(4):
                            nc.tensor.matmul(
                                pso_ap(half, jb),
                                lhsT=rt[:, 128 * jb : 128 * (jb + 1)],
                                rhs=bshl[k][:, 64 - rr : 96 - rr],
                                start=(o == 0 and k == 0 and r == 0),
                                stop=False,
                            )
                if rq is None:
                    rq = rqpool.tile([128, N], fp16, name="rq", tag="rq")
                    relu_tile(next_eng(), rq, hx[3], h3col(4 * half + o))
                for jb in range(4):
                    nc.tensor.matmul(
                        pso_ap(half, jb),
                        lhsT=rq[:, 128 * jb : 128 * (jb + 1)],
                        rhs=b3t[:, 24 - 8 * o : 56 - 8 * o],
                        start=False, stop=(o == NH // 8 - 1),
                    )

            epi = {}

            def emit_epilogue_a(half):
                # ACT + PE part: exp/ln tiles + row-sum matmuls
                e2 = epool.tile([128, 128], fp16, name="e2", tag="e2")
                t1 = epool.tile([128, 128], fp16, name="t1", tag="t1")
                # E^T = exp(s + b2): ONE activation over all 4 j-blocks
                # via a strided AP on the quad tile ([bank, 4] middle dim)
                qap = qso[half][:, 0:2048]
                e_in = type(qap)(
                    qap.tensor, qap.offset,
                    [list(qap.ap[0]), [512, 4], [1, NH]],
                )
                nc.scalar.activation(
                    out=e2, in_=e_in, func=AF.Exp, bias=b2t, scale=1.0,
                )
                # T1^T = log(1 + E^T)
                nc.scalar.activation(
                    out=t1, in_=e2, func=AF.Ln, bias=onef, scale=1.0,
                )
                # row sums over j via ones-vector matmuls (free size 1);
                # accumulate into bank 3 of the quad (after jb3's cols,
                # groups sequential per bank)
                po = qso[half][:NH, 1568:1570]
                for jb in range(4):
                    nc.tensor.matmul(
                        po[:, 0:1], lhsT=e2[:, 32 * jb : 32 * (jb + 1)],
                        rhs=one16, start=(jb == 0), stop=(jb == 3),
                    )
                for jb in range(4):
                    nc.tensor.matmul(
                        po[:, 1:2], lhsT=t1[:, 32 * jb : 32 * (jb + 1)],
                        rhs=one16, start=(jb == 0), stop=(jb == 3),
                    )
                epi[half] = (e2, po)

            def emit_epilogue_b(half):
                # DVE + DMA part: diag, out3 assembly, output DMA
                e2, po = epi[half]
                osl = slice(half * NH, (half + 1) * NH)
                tmp = epool.tile([NL, NH], fp16, name="tmp", tag="tmp")
                # ed = diag(E): rotated x puts the diag block at
                # j in [32h, 32h+32), i.e. partitions 32h.. of j-block 0
                nc.vector.tensor_tensor(
                    out=tmp[osl, :],
                    in0=e2[half * NH : (half + 1) * NH, 0:NH],
                    in1=maskh[half],
                    op=ALU.mult,
                )
                nc.vector.reduce_sum(
                    out=out3[osl, 2:3], in_=tmp[osl, :],
                    axis=mybir.AxisListType.X,
                )
                nc.vector.tensor_copy(out=out3[osl, 0:2], in_=po[:, 0:2])
                nc.sync.dma_start(out=out_o[osl, :], in_=out3[osl, :])

            for o in range(NH // 8):
                emit_oct(0, o)
            for o in range(NH // 8):
                emit_oct(1, o)
                if o == 0:
                    emit_epilogue_a(0)
                if o == 2:
                    emit_epilogue_b(0)
            emit_epilogue_a(1)
            emit_epilogue_b(1)
            psq_cm.__exit__(None, None, None)

    nc.compile()
    return nc


def _make_in_maps(x, y, W1, b1, W2):
    f16 = np.float16
    slabA = np.zeros((401, ATOT), f16)
    slabB = np.zeros((401, BTOT), f16)
    w1xT = W1[:, :D].T.astype(f16)       # [D(k), H(m)]
    w1yT = W1[:, D:].T.astype(f16)
    slabA[:D, CW1X : CW1X + 384] = w1xT[:, :384]
    slabA[:D, CW1X + 384 : CW1X + 512] = np.tile(w1xT[:, 384:400], (1, 8))
    slabB[:D, CW1Y : CW1Y + 384] = w1yT[:, :384]
    slabB[400, CW1Y : CW1Y + 384] = b1[:384].astype(f16)
    # parity blocks: [384:416) = [w1y3 | 0], [400:432) = [0 | w1y3]
    slabB[:D, CW1Y + 384 : CW1Y + 400] = w1yT[:, 384:400]
    slabB[400, CW1Y + 384 : CW1Y + 400] = b1[384:400].astype(f16)
    slabB[:D, CW1Y + 416 : CW1Y + 432] = w1yT[:, 384:400]
    slabB[400, CW1Y + 416 : CW1Y + 432] = b1[384:400].astype(f16)

    consts = np.zeros((128, CTOT), f16)
    for k in range(3):
        consts[:128, CBSH + 128 * k + 64] = W2[0, 128 * k : 128 * (k + 1)].astype(f16)
    p = np.arange(128)
    consts[p, CB3 + 24 + 2 * (p // 32) + (p % 32) // 16] = W2[0, 384 + (p % 16)].astype(f16)
    consts[:NH, CMSK : CMSK + NH] = np.eye(NH, dtype=f16)
    consts[NH : 2 * NH, CMSK : CMSK + NH] = np.eye(NH, dtype=f16)

    xT = x.T.astype(f16)
    in_maps = []
    for c in range(NCORES):
        sa = slabA.copy()
        # rotate x columns so core c's diag block lands at columns [0, 64)
        sa[:D, CX : CX + N] = np.roll(xT, -c * NL, axis=1)
        sb = slabB.copy()
        sb[:D, CYT : CYT + NL] = y[c * NL : (c + 1) * NL, :].T.astype(f16)
        sb[400, CYT : CYT + NL] = 1.0
        in_maps.append({"slabA": sa, "slabB": sb, "constd": consts})
    return in_maps


def _combine(results):
    rr = np.concatenate([r["out_o"][:, 0].astype(np.float64) for r in results])
    rs = np.concatenate([r["out_o"][:, 1].astype(np.float64) for r in results])
    ed = np.concatenate([r["out_o"][:, 2].astype(np.float64) for r in results])
    lse = np.log(np.float64(N) + rr)
    t0 = np.log1p(ed)
    t0_mean = t0.mean()
    lower = t0_mean - (lse.mean() - np.log(np.float64(N)))
    upper = t0_mean - rs.mean() / N
    return np.float32(lower), np.float32(upper)


def kernel(x_samples, y_samples, W1, b1, W2, b2, _trace=False):
    from concourse.bass_utils import run_bass_kernel_spmd

    nc = _build_program(float(np.float32(b2[0])))
    in_maps = _make_in_maps(
        np.asarray(x_samples, np.float32),
        np.asarray(y_samples, np.float32),
        np.asarray(W1, np.float32),
        np.asarray(b1, np.float32),
        np.asarray(W2, np.float32),
    )
    res = run_bass_kernel_spmd(
        nc, in_maps, core_ids=list(range(NCORES)), trace=_trace
    )
    out = _combine(res.results)
    if _trace:
        return out, res
    return out
